# revision 5
# baseline (speedup 1.0000x reference)
"""Trainium2 Bass kernel for causal self-attention with clipped softmax.

Problem (hardcoded): B=2, S=2048, H=16, D=128, fp32 inputs.
    scores = (Q @ K^T) / sqrt(D), causal mask, p = softmax(scores)
    p = clip(1.06*p - 0.03, 0, 1)            # ZETA=1.03, GAMMA=-0.03
    out = p @ V

Sharding: 32 (batch, head) pairs -> 4 per core across 8 cores (tensor
parallel over heads + data parallel over batch). No cross-core comms.

Per-core device algorithm (transposed-scores layout, bf16 matmuls):
  - Q^T, K^T are staged host-side as [NP, D, S] so no on-device transposes;
    V is staged pair-interleaved [S, NP*D] for wide DMA descriptors.
  - scoresT[k, q] strips (causal only) are packed into one contiguous
    "stream" per pair so each exp activation covers a full PSUM tile.
  - E = exp(s/sqrt(D) + ln 1.06), diagonal masked by affine_select.
  - Z^T[q, tile] via tiny-output matmuls: E-block stationary, ones column
    moving -> PSUM accumulates the per-tile softmax denominators.
  - r[q] = (1.06/0.03)/Z' via one Reciprocal activation; transposed to a
    row with DVE 32x32 stream transposes; broadcast across partitions by
    GPSIMD partition_broadcast -> rbt[k, q] tile.
  - Ehat = E * rbt (one DVE tensor_tensor pass), then ONE dual-op
    tensor_scalar clamp: t2 = max(min(Ehat, 34.333), 1.0).
    Identity: clip(1.06p - 0.03, 0, 1) = 0.03*(t2 - 1), and masked
    positions (Ehat=0 -> t2=1) plus the -1 offset are together equal to
    the causal-prefix V sums, which the HOST subtracts after the fact:
      out[d, q in tile T] = 0.03*(PV(t2)[d, q] - S_T[d]),
      S_T[d] = sum of V[k, d] over all k-tiles <= T.
  - PV(t2) accumulated in PSUM over k-tiles, drained f32 to SBUF, DMA'd
    out as [NP, D, S] f32; host applies the S_T correction + 0.03 scale.
"""

import ml_dtypes
import numpy as np

import concourse.bass as bass
import concourse.mybir as mybir
import concourse.tile as tile
from concourse import bacc
from concourse.bass_utils import run_bass_kernel_spmd

B = 2
S = 2048
H = 16
D = 128
N_CORES = 8
NP = H * B // N_CORES  # (b,h) pairs per core = 4
NT = S // 128  # 128-col tiles along sequence = 16
INV_SQRT_D = 1.0 / np.sqrt(np.float64(D))
ZETA = 1.03
GAMMA = -0.03
ALPHA = ZETA - GAMMA  # 1.06
CHI = (1.0 - GAMMA) / (-GAMMA)  # 34.3333: upper clamp for Ehat
STREAM = S * NT - 64 * NT * (NT - 1)  # 17408 packed causal columns

F32 = mybir.dt.float32
BF16 = mybir.dt.bfloat16

# strip kk (k-tile) covers q in [128*kk, S); stream offset of each strip
W_STRIP = [S - 128 * kk for kk in range(NT)]
OFF_STRIP = [0] * NT
for _kk in range(1, NT):
    OFF_STRIP[_kk] = OFF_STRIP[_kk - 1] + W_STRIP[_kk - 1]

# exp/psum chunking of the packed stream: alternate 1536/1024 tiles
CHUNKS = []  # (stream_lo, stream_hi, which_pool)
_pos = 0
_tog = 0
while _pos < STREAM:
    cw = 1536 if _tog == 0 else 1024
    hi = min(_pos + cw, STREAM)
    CHUNKS.append((_pos, hi, _tog))
    _pos = hi
    _tog ^= 1


def _strip_of(pos):
    for kk in range(NT - 1, -1, -1):
        if pos >= OFF_STRIP[kk]:
            return kk
    raise AssertionError


def build_core_program():
    nc = bacc.Bacc(
        "TRN2", target_bir_lowering=False, debug=False, num_devices=N_CORES
    )

    qt_d = nc.dram_tensor("qt", [NP, D, S], BF16, kind="ExternalInput").ap()
    kt_d = nc.dram_tensor("kt", [NP, D, S], BF16, kind="ExternalInput").ap()
    v_d = nc.dram_tensor("v", [S, NP * D], BF16, kind="ExternalInput").ap()
    ot_d = nc.dram_tensor("ot", [NP, D, S], F32, kind="ExternalOutput").ap()

    with tile.TileContext(nc) as tc:
        Builder(tc, qt_d, kt_d, v_d, ot_d).build()

    nc.compile()
    return nc


class Builder:
    def __init__(self, tc, qt_d, kt_d, v_d, ot_d):
        self.tc = tc
        self.nc = tc.nc
        self.qt_d, self.kt_d, self.v_d, self.ot_d = qt_d, kt_d, v_d, ot_d
        self.qt = [None] * NP
        self.kt = [None] * NP
        self.et = [None] * NP
        self.rbt = [None] * NP
        self.psz = [None] * NP
        self.osb = [None] * NP

    def build(self):
        nc = self.nc
        with (
            self.tc.tile_pool(name="const", bufs=1) as constp,
            self.tc.tile_pool(name="vt", bufs=1) as vtp,
            self.tc.tile_pool(name="qk", bufs=2) as qkp,
            self.tc.tile_pool(name="et", bufs=2) as etp,
            self.tc.tile_pool(name="rz", bufs=2) as rzp,
            self.tc.tile_pool(name="rb", bufs=2) as rbp,
            self.tc.tile_pool(name="ob", bufs=2) as obp,
            self.tc.tile_pool(name="psA", bufs=1, space="PSUM") as psA,
            self.tc.tile_pool(name="psB", bufs=1, space="PSUM") as psB,
            self.tc.tile_pool(name="psPV", bufs=2, space="PSUM") as psPV,
            self.tc.tile_pool(name="psZ", bufs=1, space="PSUM") as psZ,
        ):
            self.qkp, self.etp, self.rzp, self.rbp, self.obp = (
                qkp, etp, rzp, rbp, obp,
            )
            self.psA, self.psB, self.psPV, self.psZ = psA, psB, psPV, psZ
            self.vtp = vtp

            self.ones_blk = constp.tile([128, 512], BF16)
            nc.vector.memset(self.ones_blk[:], 1.0)
            self.bias_ln = constp.tile([128, 1], F32)
            nc.vector.memset(self.bias_ln[:], float(np.log(ALPHA)))

            # PE p-state warmup: keep the PE busy from t=0 so the clock is
            # fully ramped (>3us continuous) when real matmuls arrive.
            wps = self.psA.tile([128, 1536], F32, tag="schunk")
            for i in range(11):
                nc.tensor.matmul(
                    wps[:, (i % 3) * 512:(i % 3) * 512 + 512],
                    lhsT=self.ones_blk[:, 0:128],
                    rhs=self.ones_blk[:],
                    start=True, stop=True,
                )

            self.stage_in_v()
            self.stage_in(0)
            self.stage_in(1)
            self.stage_qk(0)
            self.stage_zfin(0)
            self.stage_qk(1)
            self.stage_clip(0)
            self.stage_zfin(1)
            self.stage_in(2)
            self.stage_qk(2, pv_pair=0)
            self.stage_clip(1)
            self.stage_zfin(2)
            self.stage_in(3)
            self.stage_qk(3, pv_pair=1)
            self.stage_clip(2)
            self.stage_zfin(3)
            self.stage_pv(2)
            self.stage_clip(3)
            self.stage_pv(3)

    def stage_in_v(self):
        # one wide DMA for all pairs' V: [S, NP*D] -> [128, (T, NP*D)]
        self.vt = self.vtp.tile([128, NT * NP * D], BF16)
        self.nc.sync.dma_start(
            out=self.vt[:].rearrange("p (t x) -> p t x", x=NP * D),
            in_=self.v_d.rearrange("(t p) x -> p t x", p=128),
        )

    def stage_in(self, j):
        nc = self.nc
        qt = self.qkp.tile([128, S], BF16, tag="qt")
        kt = self.qkp.tile([128, S], BF16, tag="kt")
        nc.sync.dma_start(out=kt[:], in_=self.kt_d[j])
        nc.sync.dma_start(out=qt[:], in_=self.qt_d[j])
        self.qt[j], self.kt[j] = qt, kt

    def _vblk(self, j, kk):
        off = (kk * NP + j) * D
        return self.vt[:, off:off + D]

    def stage_qk(self, j, pv_pair=None):
        """QK^T strips packed into alternating PSUM chunks, exp, diag mask,
        and mini-Z accumulation. Optionally interleaves another pair's PV
        groups between chunks to keep PE/DVE overlapped."""
        nc = self.nc
        qt, kt = self.qt[j], self.kt[j]
        et = self.etp.tile([128, STREAM], BF16, tag="et")
        psz = self.psZ.tile([128, NT], F32, tag="zt")
        self.et[j] = et
        self.psz[j] = psz

        pv_groups = list(range(4)) if pv_pair is not None else []
        pv_every = max(1, len(CHUNKS) // (len(pv_groups) + 1)) if pv_groups else 0

        for ci, (lo, hi, tog) in enumerate(CHUNKS):
            pool = self.psA if tog == 0 else self.psB
            width = 1536 if tog == 0 else 1024
            ps = pool.tile([128, width], F32, tag="schunk")
            # matmul pieces: split at strip boundaries and 512-grid of tile
            pos = lo
            while pos < hi:
                kk = _strip_of(pos)
                strip_end = OFF_STRIP[kk] + W_STRIP[kk]
                seg_end = min(hi, strip_end, lo + ((pos - lo) // 512 + 1) * 512)
                qa = 128 * kk + (pos - OFF_STRIP[kk])
                qb = qa + (seg_end - pos)
                nc.tensor.matmul(
                    ps[:, pos - lo:seg_end - lo],
                    lhsT=kt[:, 128 * kk:128 * kk + 128],
                    rhs=qt[:, qa:qb],
                    start=True, stop=True,
                )
                pos = seg_end
            nc.scalar.activation(
                et[:, lo:hi],
                ps[:, 0:hi - lo],
                mybir.ActivationFunctionType.Exp,
                scale=float(INV_SQRT_D),
                bias=self.bias_ln[:],
            )
            # diagonal masks + mini-Z for every diag block inside this chunk
            for kk in range(NT):
                dlo = OFF_STRIP[kk]
                if lo <= dlo and dlo + 128 <= hi:
                    nc.gpsimd.affine_select(
                        out=et[:, dlo:dlo + 128],
                        in_=et[:, dlo:dlo + 128],
                        compare_op=mybir.AluOpType.is_ge,
                        fill=0.0,
                        base=0,
                        pattern=[[1, 128]],
                        channel_multiplier=-1,
                    )
                    # Z^T column kk: sum E over k across strips 0..kk at
                    # q-tile kk (each strip's block is ready: the diagonal
                    # is always the last-placed block in stream order)
                    T = kk
                    for k2 in range(T + 1):
                        blk = OFF_STRIP[k2] + 128 * (T - k2)
                        nc.tensor.matmul(
                            psz[:, T:T + 1],
                            lhsT=et[:, blk:blk + 128],
                            rhs=self.ones_blk[:, 0:1],
                            start=(k2 == 0), stop=(k2 == T),
                        )
            if pv_groups and pv_every and (ci + 1) % pv_every == 0:
                self.emit_pv_group(pv_pair, pv_groups.pop(0))
        for g in pv_groups:
            self.emit_pv_group(pv_pair, g)
        if pv_pair is not None:
            self.finish_pv(pv_pair)

    def stage_zfin(self, j):
        """Reciprocal + transpose to row + partition broadcast -> rbt."""
        nc = self.nc
        rt32 = self.rzp.tile([128, 16], F32, tag="rt32")
        rt = self.rzp.tile([128, 32], BF16, tag="rt")
        rrow = self.rzp.tile([32, 128], BF16, tag="rrow")
        rbt = self.rbp.tile([128, S], BF16, tag="rbt")
        self.rbt[j] = rbt
        # r = (ALPHA/0.03)/Z'  (constant folded into the bf16 cast below)
        nc.vector.reciprocal(rt32[:, 0:NT], self.psz[j][:, 0:NT])
        nc.vector.tensor_scalar_mul(rt[:, 0:NT], rt32[:, 0:NT], float(ALPHA / -GAMMA))
        nc.vector.memset(rt[:, NT:32], 1.0)
        for i in range(4):
            nc.vector.transpose(
                out=rrow[0:32, 32 * i:32 * i + 32],
                in_=rt[32 * i:32 * i + 32, 0:32],
            )
        # gather the 16 transposed rows into one q-major row on partition 0,
        # then broadcast it across all partitions in a single Pool op
        row0 = self.rzp.tile([1, S], BF16, tag="row0")
        nc.sync.dma_start(out=row0[0:1, 0:S], in_=rrow[0:NT, 0:128])
        nc.gpsimd.partition_broadcast(rbt[:], row0[0:1, 0:S], channels=128)

    def stage_clip(self, j):
        """Ehat = E * rbt (per strip), then one dual-op clamp over the
        whole packed stream: t2 = max(min(Ehat, CHI), 1)."""
        nc = self.nc
        et, rbt = self.et[j], self.rbt[j]
        for kk in range(NT):
            lo, w = OFF_STRIP[kk], W_STRIP[kk]
            nc.vector.tensor_tensor(
                et[:, lo:lo + w],
                et[:, lo:lo + w],
                rbt[:, 128 * kk:S],
                mybir.AluOpType.mult,
            )
        nc.vector.tensor_scalar(
            et[:], et[:], float(CHI), 1.0,
            mybir.AluOpType.min, mybir.AluOpType.max,
        )

    def emit_pv_group(self, j, g):
        nc = self.nc
        et = self.et[j]
        if self.osb[j] is None:
            self.osb[j] = self.obp.tile([128, S], F32, tag="osb", name="osb")
        osb = self.osb[j]
        glo, ghi = 512 * g, 512 * (g + 1)
        kmax = 4 * g + 3
        ps = self.psPV.tile([128, 512], F32, tag="pv")
        for kk in range(kmax + 1):
            qlo = max(glo, 128 * kk)
            src = OFF_STRIP[kk] + (qlo - 128 * kk)
            nc.tensor.matmul(
                ps[:, qlo - glo:512],
                lhsT=self._vblk(j, kk),
                rhs=et[:, src:src + (ghi - qlo)],
                start=(kk == 0), stop=(kk == kmax),
            )
        nc.vector.tensor_scalar_add(osb[:, glo:ghi], ps[:, 0:512], 0.0)

    def finish_pv(self, j):
        self.nc.sync.dma_start(out=self.ot_d[j], in_=self.osb[j][:])

    def stage_pv(self, j):
        for g in range(4):
            self.emit_pv_group(j, g)
        self.finish_pv(j)


_NC_CACHE = None


def _get_program():
    global _NC_CACHE
    if _NC_CACHE is None:
        _NC_CACHE = build_core_program()
    return _NC_CACHE


def kernel(query_states, key_states, value_states, batch_size, q_length, kv_length):
    assert int(batch_size) == B and int(q_length) == S and int(kv_length) == S
    qf = np.asarray(query_states, dtype=np.float32).reshape(B, S, H, D)
    kf = np.asarray(key_states, dtype=np.float32).reshape(B, S, H, D)
    vf = np.asarray(value_states, dtype=np.float32).reshape(B, S, H, D)

    nc = _get_program()

    in_maps = []
    s_host = []  # per core: [NP, NT, D] causal-prefix sums of bf16 V
    for c in range(N_CORES):
        b = c // (N_CORES // B)
        h0 = NP * (c % (N_CORES // B))
        qb = qf[b, :, h0:h0 + NP, :].astype(ml_dtypes.bfloat16)  # [S, NP, D]
        kb = kf[b, :, h0:h0 + NP, :].astype(ml_dtypes.bfloat16)
        vb = vf[b, :, h0:h0 + NP, :].astype(ml_dtypes.bfloat16)
        in_maps.append(
            {
                "qt": np.ascontiguousarray(qb.transpose(1, 2, 0)),  # [NP,D,S]
                "kt": np.ascontiguousarray(kb.transpose(1, 2, 0)),
                "v": np.ascontiguousarray(vb.reshape(S, NP * D)),
            }
        )
        # S_T[d] = sum of V over k-tiles 0..T (f32 accumulation of bf16 V)
        vt32 = vb.astype(np.float32).reshape(NT, 128, NP, D)
        s_host.append(np.cumsum(vt32.sum(axis=1), axis=0).transpose(1, 0, 2))

    res = run_bass_kernel_spmd(nc, in_maps, list(range(N_CORES)))

    out = np.empty((B, S, H, D), dtype=np.float32)
    for c in range(N_CORES):
        b = c // (N_CORES // B)
        h0 = NP * (c % (N_CORES // B))
        ot = np.asarray(res.results[c]["ot"])  # [NP, D, S] = PV(t2)
        for jj in range(NP):
            pv = ot[jj].T.reshape(NT, 128, D)  # [T, q, D]
            pv = pv - s_host[c][jj][:, None, :]
            out[b, :, h0 + jj, :] = (-GAMMA) * pv.reshape(S, D)
    return out.reshape(B * S, H, D)


# revision 8
# speedup vs baseline: 1.0477x; 1.0477x over previous
"""Trainium2 Bass kernel for causal self-attention with clipped softmax.

Problem (hardcoded): B=2, S=2048, H=16, D=128, fp32 inputs.
    scores = (Q @ K^T) / sqrt(D), causal mask, p = softmax(scores)
    p = clip(1.06*p - 0.03, 0, 1)            # ZETA=1.03, GAMMA=-0.03
    out = p @ V

Sharding: 32 (batch, head) pairs -> 4 per core across 8 cores (tensor
parallel over heads + data parallel over batch). No cross-core comms.

Per-core device algorithm (transposed-scores layout, bf16 matmuls):
  - Q^T, K^T are staged host-side as [NP, D, S] so no on-device transposes;
    V is staged pair-interleaved [S, NP*D] for wide DMA descriptors.
  - scoresT[k, q] strips (causal only) are packed into one contiguous
    "stream" per pair so each exp activation covers a full PSUM tile.
  - E = exp(s/sqrt(D) + ln 1.06), diagonal masked by affine_select.
  - Z^T[q, tile] via tiny-output matmuls: E-block stationary, ones column
    moving -> PSUM accumulates the per-tile softmax denominators.
  - r[q] = (1.06/0.03)/Z' via one Reciprocal activation; transposed to a
    row with DVE 32x32 stream transposes; broadcast across partitions by
    GPSIMD partition_broadcast -> rbt[k, q] tile.
  - Ehat = E * rbt (one DVE tensor_tensor pass), then ONE dual-op
    tensor_scalar clamp: t2 = max(min(Ehat, 34.333), 1.0).
    Identity: clip(1.06p - 0.03, 0, 1) = 0.03*(t2 - 1), and masked
    positions (Ehat=0 -> t2=1) plus the -1 offset are together equal to
    the causal-prefix V sums, which the HOST subtracts after the fact:
      out[d, q in tile T] = 0.03*(PV(t2)[d, q] - S_T[d]),
      S_T[d] = sum of V[k, d] over all k-tiles <= T.
  - PV(t2) accumulated in PSUM over k-tiles, drained f32 to SBUF, DMA'd
    out as [NP, D, S] f32; host applies the S_T correction + 0.03 scale.
"""

import ml_dtypes
import numpy as np

import concourse.bass as bass
import concourse.mybir as mybir
import concourse.tile as tile
from concourse import bacc
from concourse.bass_utils import run_bass_kernel_spmd

B = 2
S = 2048
H = 16
D = 128
N_CORES = 8
NP = H * B // N_CORES  # (b,h) pairs per core = 4
NT = S // 128  # 128-col tiles along sequence = 16
INV_SQRT_D = 1.0 / np.sqrt(np.float64(D))
ZETA = 1.03
GAMMA = -0.03
ALPHA = ZETA - GAMMA  # 1.06
CHI = (1.0 - GAMMA) / (-GAMMA)  # 34.3333: upper clamp for Ehat
STREAM = S * NT - 64 * NT * (NT - 1)  # 17408 packed causal columns

F32 = mybir.dt.float32
BF16 = mybir.dt.bfloat16

# strip kk (k-tile) covers q in [128*kk, S); stream offset of each strip
W_STRIP = [S - 128 * kk for kk in range(NT)]
OFF_STRIP = [0] * NT
for _kk in range(1, NT):
    OFF_STRIP[_kk] = OFF_STRIP[_kk - 1] + W_STRIP[_kk - 1]

# exp/psum chunking of the packed stream: alternate 1536/1024 tiles
CHUNKS = []  # (stream_lo, stream_hi, which_pool)
_pos = 0
_tog = 0
while _pos < STREAM:
    cw = 1536 if _tog == 0 else 1024
    hi = min(_pos + cw, STREAM)
    CHUNKS.append((_pos, hi, _tog))
    _pos = hi
    _tog ^= 1


def _strip_of(pos):
    for kk in range(NT - 1, -1, -1):
        if pos >= OFF_STRIP[kk]:
            return kk
    raise AssertionError


def build_core_program():
    nc = bacc.Bacc(
        "TRN2", target_bir_lowering=False, debug=False, num_devices=N_CORES
    )

    qt_d = nc.dram_tensor("qt", [NP, D, S], BF16, kind="ExternalInput").ap()
    kt_d = nc.dram_tensor("kt", [NP, D, S], BF16, kind="ExternalInput").ap()
    v_d = nc.dram_tensor("v", [S, NP * D], BF16, kind="ExternalInput").ap()
    ot_d = nc.dram_tensor("ot", [NP, D, S], F32, kind="ExternalOutput").ap()

    with tile.TileContext(nc) as tc:
        Builder(tc, qt_d, kt_d, v_d, ot_d).build()

    nc.compile()
    return nc


class Builder:
    def __init__(self, tc, qt_d, kt_d, v_d, ot_d):
        self.tc = tc
        self.nc = tc.nc
        self.qt_d, self.kt_d, self.v_d, self.ot_d = qt_d, kt_d, v_d, ot_d
        self.qt = [None] * NP
        self.kt = [None] * NP
        self.et = [None] * NP
        self.rbt = [None] * NP
        self.psz = [None] * NP
        self.osb = [None] * NP

    def build(self):
        nc = self.nc
        with (
            self.tc.tile_pool(name="const", bufs=1) as constp,
            self.tc.tile_pool(name="vt", bufs=1) as vtp,
            self.tc.tile_pool(name="qk", bufs=2) as qkp,
            self.tc.tile_pool(name="et", bufs=2) as etp,
            self.tc.tile_pool(name="rz", bufs=2) as rzp,
            self.tc.tile_pool(name="rb", bufs=2) as rbp,
            self.tc.tile_pool(name="ob", bufs=2) as obp,
            self.tc.tile_pool(name="psA", bufs=1, space="PSUM") as psA,
            self.tc.tile_pool(name="psB", bufs=1, space="PSUM") as psB,
            self.tc.tile_pool(name="psPV", bufs=2, space="PSUM") as psPV,
            self.tc.tile_pool(name="psZ", bufs=1, space="PSUM") as psZ,
        ):
            self.qkp, self.etp, self.rzp, self.rbp, self.obp = (
                qkp, etp, rzp, rbp, obp,
            )
            self.psA, self.psB, self.psPV, self.psZ = psA, psB, psPV, psZ
            self.vtp = vtp

            self.ones_blk = constp.tile([128, 512], BF16)
            nc.vector.memset(self.ones_blk[:], 1.0)
            self.bias_ln = constp.tile([128, 1], F32)
            nc.vector.memset(self.bias_ln[:], float(np.log(ALPHA)))

            # PE p-state warmup: keep the PE busy from t=0 so the clock is
            # fully ramped (>3us continuous) when real matmuls arrive.
            wps = self.psA.tile([128, 1536], F32, tag="schunk")
            for i in range(11):
                nc.tensor.matmul(
                    wps[:, (i % 3) * 512:(i % 3) * 512 + 512],
                    lhsT=self.ones_blk[:, 0:128],
                    rhs=self.ones_blk[:],
                    start=True, stop=True,
                )

            self.stage_in_v()
            self.stage_in(0)
            self.stage_in(1)
            self.stage_qk(0)
            self.stage_zfin(0)
            self.stage_qk(1, pv_pair=0)
            self.stage_zfin(1)
            self.stage_in(2)
            self.stage_qk(2, pv_pair=1)
            self.stage_zfin(2)
            self.stage_in(3)
            self.stage_qk(3, pv_pair=2)
            self.stage_zfin(3)
            self.stage_clipv(3)

    def stage_in_v(self):
        # one wide DMA for all pairs' V: [S, NP*D] -> [128, (T, NP*D)]
        self.vt = self.vtp.tile([128, NT * NP * D], BF16)
        self.nc.sync.dma_start(
            out=self.vt[:].rearrange("p (t x) -> p t x", x=NP * D),
            in_=self.v_d.rearrange("(t p) x -> p t x", p=128),
        )

    def stage_in(self, j):
        nc = self.nc
        qt = self.qkp.tile([128, S], BF16, tag="qt")
        kt = self.qkp.tile([128, S], BF16, tag="kt")
        nc.sync.dma_start(out=kt[:], in_=self.kt_d[j])
        nc.sync.dma_start(out=qt[:], in_=self.qt_d[j])
        self.qt[j], self.kt[j] = qt, kt

    def _vblk(self, j, kk):
        off = (kk * NP + j) * D
        return self.vt[:, off:off + D]

    def stage_qk(self, j, pv_pair=None):
        """QK^T strips packed into alternating PSUM chunks + exp + inline
        Pool diag masks; the mini-Z matmuls go AFTER all chunks so they
        never block the PE queue mid-stream. Optionally interleaves the
        previous pair's clip/PV steps between chunks as PE filler."""
        nc = self.nc
        qt, kt = self.qt[j], self.kt[j]
        et = self.etp.tile([128, STREAM], BF16, tag="et")
        psz = self.psZ.tile([128, NT], F32, tag="zt")
        self.et[j] = et
        self.psz[j] = psz

        steps = list(range(4)) if pv_pair is not None else []
        every = max(1, len(CHUNKS) // (len(steps) + 1)) if steps else 0

        for ci, (lo, hi, tog) in enumerate(CHUNKS):
            pool = self.psA if tog == 0 else self.psB
            width = 1536 if tog == 0 else 1024
            ps = pool.tile([128, width], F32, tag="schunk")
            # matmul pieces: split at strip boundaries and 512-grid of tile
            pos = lo
            while pos < hi:
                kk = _strip_of(pos)
                strip_end = OFF_STRIP[kk] + W_STRIP[kk]
                seg_end = min(hi, strip_end, lo + ((pos - lo) // 512 + 1) * 512)
                qa = 128 * kk + (pos - OFF_STRIP[kk])
                qb = qa + (seg_end - pos)
                nc.tensor.matmul(
                    ps[:, pos - lo:seg_end - lo],
                    lhsT=kt[:, 128 * kk:128 * kk + 128],
                    rhs=qt[:, qa:qb],
                    start=True, stop=True,
                )
                pos = seg_end
            nc.scalar.activation(
                et[:, lo:hi],
                ps[:, 0:hi - lo],
                mybir.ActivationFunctionType.Exp,
                scale=float(INV_SQRT_D),
                bias=self.bias_ln[:],
            )
            # Pool-only diagonal masks as their chunk lands
            for kk in range(NT):
                dlo = OFF_STRIP[kk]
                if lo <= dlo and dlo + 128 <= hi:
                    nc.gpsimd.affine_select(
                        out=et[:, dlo:dlo + 128],
                        in_=et[:, dlo:dlo + 128],
                        compare_op=mybir.AluOpType.is_ge,
                        fill=0.0,
                        base=0,
                        pattern=[[1, 128]],
                        channel_multiplier=-1,
                    )
            if steps and every and (ci + 1) % every == 0:
                self.emit_clipv_step(pv_pair, steps.pop(0))
        # mini-Z: Z^T column T accumulates E over strips 0..T at q-tile T
        for T in range(NT):
            for k2 in range(T + 1):
                blk = OFF_STRIP[k2] + 128 * (T - k2)
                nc.tensor.matmul(
                    psz[:, T:T + 1],
                    lhsT=et[:, blk:blk + 128],
                    rhs=self.ones_blk[:, 0:1],
                    start=(k2 == 0), stop=(k2 == T),
                )
        for g in steps:
            self.emit_clipv_step(pv_pair, g)
        if pv_pair is not None:
            self.finish_pv(pv_pair)

    def stage_zfin(self, j):
        """Reciprocal + transpose to row + partition broadcast -> rbt."""
        nc = self.nc
        rt32 = self.rzp.tile([128, 16], F32, tag="rt32")
        rt = self.rzp.tile([128, 32], BF16, tag="rt")
        rrow = self.rzp.tile([32, 128], BF16, tag="rrow")
        rbt = self.rbp.tile([128, S], BF16, tag="rbt")
        self.rbt[j] = rbt
        # r = (ALPHA/0.03)/Z'  (constant folded into the bf16 cast below)
        nc.vector.reciprocal(rt32[:, 0:NT], self.psz[j][:, 0:NT])
        nc.vector.tensor_scalar_mul(rt[:, 0:NT], rt32[:, 0:NT], float(ALPHA / -GAMMA))
        nc.vector.memset(rt[:, NT:32], 1.0)
        for i in range(4):
            nc.vector.transpose(
                out=rrow[0:32, 32 * i:32 * i + 32],
                in_=rt[32 * i:32 * i + 32, 0:32],
            )
        # gather the 16 transposed rows into one q-major row on partition 0,
        # then broadcast it across all partitions in a single Pool op
        row0 = self.rzp.tile([1, S], BF16, tag="row0")
        nc.sync.dma_start(out=row0[0:1, 0:S], in_=rrow[0:NT, 0:128])
        nc.gpsimd.partition_broadcast(rbt[:], row0[0:1, 0:S], channels=128)

    def emit_clipv_step(self, j, g):
        """Clip the strips newly needed by PV group g (Ehat multiply per
        strip + one dual-op clamp over their contiguous stream range),
        then the PV group matmuls and the PSUM drain."""
        nc = self.nc
        et, rbt = self.et[j], self.rbt[j]
        if self.osb[j] is None:
            self.osb[j] = self.obp.tile([128, S], F32, tag="osb", name="osb")
        osb = self.osb[j]
        glo, ghi = 512 * g, 512 * (g + 1)
        kmax = 4 * g + 3
        for kk in range(4 * g, kmax + 1):
            lo, w = OFF_STRIP[kk], W_STRIP[kk]
            nc.vector.tensor_tensor(
                et[:, lo:lo + w],
                et[:, lo:lo + w],
                rbt[:, 128 * kk:S],
                mybir.AluOpType.mult,
            )
        clo = OFF_STRIP[4 * g]
        chi_ = OFF_STRIP[kmax] + W_STRIP[kmax]
        nc.vector.tensor_scalar(
            et[:, clo:chi_], et[:, clo:chi_], float(CHI), 1.0,
            mybir.AluOpType.min, mybir.AluOpType.max,
        )
        ps = self.psPV.tile([128, 512], F32, tag="pv")
        for kk in range(kmax + 1):
            qlo = max(glo, 128 * kk)
            src = OFF_STRIP[kk] + (qlo - 128 * kk)
            nc.tensor.matmul(
                ps[:, qlo - glo:512],
                lhsT=self._vblk(j, kk),
                rhs=et[:, src:src + (ghi - qlo)],
                start=(kk == 0), stop=(kk == kmax),
            )
        nc.vector.tensor_scalar_add(osb[:, glo:ghi], ps[:, 0:512], 0.0)

    def finish_pv(self, j):
        self.nc.sync.dma_start(out=self.ot_d[j], in_=self.osb[j][:])

    def stage_clipv(self, j):
        for g in range(4):
            self.emit_clipv_step(j, g)
        self.finish_pv(j)


_NC_CACHE = None


def _get_program():
    global _NC_CACHE
    if _NC_CACHE is None:
        _NC_CACHE = build_core_program()
    return _NC_CACHE


def kernel(query_states, key_states, value_states, batch_size, q_length, kv_length):
    assert int(batch_size) == B and int(q_length) == S and int(kv_length) == S
    qf = np.asarray(query_states, dtype=np.float32).reshape(B, S, H, D)
    kf = np.asarray(key_states, dtype=np.float32).reshape(B, S, H, D)
    vf = np.asarray(value_states, dtype=np.float32).reshape(B, S, H, D)

    nc = _get_program()

    in_maps = []
    s_host = []  # per core: [NP, NT, D] causal-prefix sums of bf16 V
    for c in range(N_CORES):
        b = c // (N_CORES // B)
        h0 = NP * (c % (N_CORES // B))
        qb = qf[b, :, h0:h0 + NP, :].astype(ml_dtypes.bfloat16)  # [S, NP, D]
        kb = kf[b, :, h0:h0 + NP, :].astype(ml_dtypes.bfloat16)
        vb = vf[b, :, h0:h0 + NP, :].astype(ml_dtypes.bfloat16)
        in_maps.append(
            {
                "qt": np.ascontiguousarray(qb.transpose(1, 2, 0)),  # [NP,D,S]
                "kt": np.ascontiguousarray(kb.transpose(1, 2, 0)),
                "v": np.ascontiguousarray(vb.reshape(S, NP * D)),
            }
        )
        # S_T[d] = sum of V over k-tiles 0..T (f32 accumulation of bf16 V)
        vt32 = vb.astype(np.float32).reshape(NT, 128, NP, D)
        s_host.append(np.cumsum(vt32.sum(axis=1), axis=0).transpose(1, 0, 2))

    res = run_bass_kernel_spmd(nc, in_maps, list(range(N_CORES)))

    out = np.empty((B, S, H, D), dtype=np.float32)
    for c in range(N_CORES):
        b = c // (N_CORES // B)
        h0 = NP * (c % (N_CORES // B))
        ot = np.asarray(res.results[c]["ot"])  # [NP, D, S] = PV(t2)
        for jj in range(NP):
            pv = ot[jj].T.reshape(NT, 128, D)  # [T, q, D]
            pv = pv - s_host[c][jj][:, None, :]
            out[b, :, h0 + jj, :] = (-GAMMA) * pv.reshape(S, D)
    return out.reshape(B * S, H, D)


# revision 12
# speedup vs baseline: 1.0483x; 1.0006x over previous
"""Trainium2 Bass kernel for causal self-attention with clipped softmax.

Problem (hardcoded): B=2, S=2048, H=16, D=128, fp32 inputs.
    scores = (Q @ K^T) / sqrt(D), causal mask, p = softmax(scores)
    p = clip(1.06*p - 0.03, 0, 1)            # ZETA=1.03, GAMMA=-0.03
    out = p @ V

Sharding: 32 (batch, head) pairs -> 4 per core across 8 cores (tensor
parallel over heads + data parallel over batch). No cross-core comms.

Per-core device algorithm (transposed-scores layout, bf16 matmuls):
  - Q^T, K^T are staged host-side as [NP, D, S] so no on-device transposes;
    V is staged pair-interleaved [S, NP*D] for wide DMA descriptors.
  - scoresT[k, q] strips (causal only) are packed into one contiguous
    "stream" per pair so each exp activation covers a full PSUM tile.
  - E = exp(s/sqrt(D) + ln 1.06), diagonal masked by affine_select.
  - Z^T[q, tile] via tiny-output matmuls: E-block stationary, ones column
    moving -> PSUM accumulates the per-tile softmax denominators.
  - r[q] = (1.06/0.03)/Z' via one Reciprocal activation; transposed to a
    row with DVE 32x32 stream transposes; broadcast across partitions by
    GPSIMD partition_broadcast -> rbt[k, q] tile.
  - Ehat = E * rbt (one DVE tensor_tensor pass), then ONE dual-op
    tensor_scalar clamp: t2 = max(min(Ehat, 34.333), 1.0).
    Identity: clip(1.06p - 0.03, 0, 1) = 0.03*(t2 - 1), and masked
    positions (Ehat=0 -> t2=1) plus the -1 offset are together equal to
    the causal-prefix V sums, which the HOST subtracts after the fact:
      out[d, q in tile T] = 0.03*(PV(t2)[d, q] - S_T[d]),
      S_T[d] = sum of V[k, d] over all k-tiles <= T.
  - PV(t2) accumulated in PSUM over k-tiles, drained f32 to SBUF, DMA'd
    out as [NP, D, S] f32; host applies the S_T correction + 0.03 scale.
"""

import ml_dtypes
import numpy as np

import concourse.bass as bass
import concourse.mybir as mybir
import concourse.tile as tile
from concourse import bacc
from concourse.bass_utils import run_bass_kernel_spmd

B = 2
S = 2048
H = 16
D = 128
N_CORES = 8
NP = H * B // N_CORES  # (b,h) pairs per core = 4
NT = S // 128  # 128-col tiles along sequence = 16
INV_SQRT_D = 1.0 / np.sqrt(np.float64(D))
ZETA = 1.03
GAMMA = -0.03
ALPHA = ZETA - GAMMA  # 1.06
CHI = (1.0 - GAMMA) / (-GAMMA)  # 34.3333: upper clamp for Ehat
STREAM = S * NT - 64 * NT * (NT - 1)  # 17408 packed causal columns

F32 = mybir.dt.float32
BF16 = mybir.dt.bfloat16

# strip kk (k-tile) covers q in [128*kk, S); stream offset of each strip
W_STRIP = [S - 128 * kk for kk in range(NT)]
OFF_STRIP = [0] * NT
for _kk in range(1, NT):
    OFF_STRIP[_kk] = OFF_STRIP[_kk - 1] + W_STRIP[_kk - 1]

# exp/psum chunking of the packed stream: alternate 1536/1024 tiles
CHUNKS = []  # (stream_lo, stream_hi, which_pool)
_pos = 0
_tog = 0
while _pos < STREAM:
    cw = 1536 if _tog == 0 else 1024
    hi = min(_pos + cw, STREAM)
    CHUNKS.append((_pos, hi, _tog))
    _pos = hi
    _tog ^= 1


def _strip_of(pos):
    for kk in range(NT - 1, -1, -1):
        if pos >= OFF_STRIP[kk]:
            return kk
    raise AssertionError


def build_core_program():
    nc = bacc.Bacc(
        "TRN2", target_bir_lowering=False, debug=False, num_devices=N_CORES
    )

    qt_d = nc.dram_tensor("qt", [NP, D, S], BF16, kind="ExternalInput").ap()
    kt_d = nc.dram_tensor("kt", [NP, D, S], BF16, kind="ExternalInput").ap()
    v_d = nc.dram_tensor("v", [S, NP * D], BF16, kind="ExternalInput").ap()
    ot_d = nc.dram_tensor("ot", [NP, D, S], F32, kind="ExternalOutput").ap()

    with tile.TileContext(nc) as tc:
        Builder(tc, qt_d, kt_d, v_d, ot_d).build()

    nc.compile()
    return nc


class Builder:
    def __init__(self, tc, qt_d, kt_d, v_d, ot_d):
        self.tc = tc
        self.nc = tc.nc
        self.qt_d, self.kt_d, self.v_d, self.ot_d = qt_d, kt_d, v_d, ot_d
        self.qt = [None] * NP
        self.kt = [None] * NP
        self.et = [None] * NP
        self.rbt = [None] * NP
        self.psz = [None] * NP
        self.osb = [None] * NP

    def build(self):
        nc = self.nc
        with (
            self.tc.tile_pool(name="const", bufs=1) as constp,
            self.tc.tile_pool(name="vt", bufs=1) as vtp,
            self.tc.tile_pool(name="qk", bufs=2) as qkp,
            self.tc.tile_pool(name="et", bufs=2) as etp,
            self.tc.tile_pool(name="rz", bufs=2) as rzp,
            self.tc.tile_pool(name="rb", bufs=2) as rbp,
            self.tc.tile_pool(name="ob", bufs=2) as obp,
            self.tc.tile_pool(name="psA", bufs=1, space="PSUM") as psA,
            self.tc.tile_pool(name="psB", bufs=1, space="PSUM") as psB,
            self.tc.tile_pool(name="psPV", bufs=2, space="PSUM") as psPV,
            self.tc.tile_pool(name="psZ", bufs=1, space="PSUM") as psZ,
        ):
            self.qkp, self.etp, self.rzp, self.rbp, self.obp = (
                qkp, etp, rzp, rbp, obp,
            )
            self.psA, self.psB, self.psPV, self.psZ = psA, psB, psPV, psZ
            self.vtp = vtp

            self.ones_blk = constp.tile([128, 512], BF16)
            nc.vector.memset(self.ones_blk[:], 1.0)
            self.bias_ln = constp.tile([128, 1], F32)
            nc.vector.memset(self.bias_ln[:], float(np.log(ALPHA)))

            # PE p-state warmup: keep the PE busy from t=0 so the clock is
            # fully ramped (>3us continuous) when real matmuls arrive.
            wps = self.psA.tile([128, 1536], F32, tag="schunk")
            for i in range(6):
                nc.tensor.matmul(
                    wps[:, (i % 3) * 512:(i % 3) * 512 + 512],
                    lhsT=self.ones_blk[:, 0:128],
                    rhs=self.ones_blk[:],
                    start=True, stop=True,
                )

            self.stage_in(0)
            self.stage_in(1)
            self.stage_in_v()
            self.stage_qk(0)
            self.stage_zfin(0)
            self.stage_clip(0)
            self.stage_in(2)
            self.stage_qk(1, pv_pair=0)
            self.stage_zfin(1)
            self.stage_clip(1)
            self.stage_in(3)
            self.stage_qk(2, pv_pair=1)
            self.stage_zfin(2)
            self.stage_clip(2)
            self.stage_qk(3, pv_pair=2)
            self.stage_zfin(3)
            self.stage_clip(3, interleave_pv=True)
            self.emit_drains(3)
            self.finish_pv(3)

    def stage_in_v(self):
        # one wide DMA for all pairs' V: [S, NP*D] -> [128, (T, NP*D)]
        self.vt = self.vtp.tile([128, NT * NP * D], BF16)
        self.nc.sync.dma_start(
            out=self.vt[:].rearrange("p (t x) -> p t x", x=NP * D),
            in_=self.v_d.rearrange("(t p) x -> p t x", p=128),
        )

    def stage_in(self, j):
        nc = self.nc
        qt = self.qkp.tile([128, S], BF16, tag="qt")
        kt = self.qkp.tile([128, S], BF16, tag="kt")
        nc.sync.dma_start(out=kt[:], in_=self.kt_d[j])
        nc.sync.dma_start(out=qt[:], in_=self.qt_d[j])
        self.qt[j], self.kt[j] = qt, kt

    def _vblk(self, j, kk):
        off = (kk * NP + j) * D
        return self.vt[:, off:off + D]

    def stage_qk(self, j, pv_pair=None):
        """QK^T strips packed into alternating PSUM chunks + exp + inline
        Pool diag masks; the mini-Z matmuls go AFTER all chunks so they
        never block the PE queue mid-stream. Optionally interleaves the
        previous pair's clip/PV steps between chunks as PE filler."""
        nc = self.nc
        qt, kt = self.qt[j], self.kt[j]
        et = self.etp.tile([128, STREAM], BF16, tag="et")
        psz = self.psZ.tile([128, NT], F32, tag="zt")
        self.et[j] = et
        self.psz[j] = psz

        # fillers: previous pair's PV groups at late chunk slots (after its
        # clip has had time on DVE), drains + out-DMA at the very end
        fillers = {}
        if pv_pair is not None:
            fillers = {
                4: lambda: self.emit_pv_group(pv_pair, 0),
                6: lambda: self.emit_pv_group(pv_pair, 1),
                8: lambda: self.emit_pv_group(pv_pair, 2),
                10: lambda: self.emit_pv_group(pv_pair, 3),
                11: lambda: (self.emit_drains(pv_pair),
                             self.finish_pv(pv_pair)),
            }

        for ci, (lo, hi, tog) in enumerate(CHUNKS):
            pool = self.psA if tog == 0 else self.psB
            width = 1536 if tog == 0 else 1024
            ps = pool.tile([128, width], F32, tag="schunk")
            # matmul pieces: split at strip boundaries and 512-grid of tile
            pos = lo
            while pos < hi:
                kk = _strip_of(pos)
                strip_end = OFF_STRIP[kk] + W_STRIP[kk]
                seg_end = min(hi, strip_end, lo + ((pos - lo) // 512 + 1) * 512)
                qa = 128 * kk + (pos - OFF_STRIP[kk])
                qb = qa + (seg_end - pos)
                nc.tensor.matmul(
                    ps[:, pos - lo:seg_end - lo],
                    lhsT=kt[:, 128 * kk:128 * kk + 128],
                    rhs=qt[:, qa:qb],
                    start=True, stop=True,
                )
                pos = seg_end
            nc.scalar.activation(
                et[:, lo:hi],
                ps[:, 0:hi - lo],
                mybir.ActivationFunctionType.Exp,
                scale=float(INV_SQRT_D),
                bias=self.bias_ln[:],
            )
            # Pool-only diagonal masks as their chunk lands
            for kk in range(NT):
                dlo = OFF_STRIP[kk]
                if lo <= dlo and dlo + 128 <= hi:
                    nc.gpsimd.affine_select(
                        out=et[:, dlo:dlo + 128],
                        in_=et[:, dlo:dlo + 128],
                        compare_op=mybir.AluOpType.is_ge,
                        fill=0.0,
                        base=0,
                        pattern=[[1, 128]],
                        channel_multiplier=-1,
                    )
            if ci in fillers:
                fillers.pop(ci)()
        # mini-Z: Z^T column T accumulates E over strips 0..T at q-tile T
        for T in range(NT):
            for k2 in range(T + 1):
                blk = OFF_STRIP[k2] + 128 * (T - k2)
                nc.tensor.matmul(
                    psz[:, T:T + 1],
                    lhsT=et[:, blk:blk + 128],
                    rhs=self.ones_blk[:, 0:1],
                    start=(k2 == 0), stop=(k2 == T),
                )
        for ci in sorted(fillers):
            fillers.pop(ci)()

    def stage_zfin(self, j):
        """Reciprocal + transpose to row + partition broadcast -> rbt."""
        nc = self.nc
        rt32 = self.rzp.tile([128, 16], F32, tag="rt32")
        rt = self.rzp.tile([128, 32], BF16, tag="rt")
        rrow = self.rzp.tile([32, 128], BF16, tag="rrow")
        rbt = self.rbp.tile([128, S], BF16, tag="rbt")
        self.rbt[j] = rbt
        # r = (ALPHA/0.03)/Z'  (constant folded into the bf16 cast below)
        nc.vector.reciprocal(rt32[:, 0:NT], self.psz[j][:, 0:NT])
        nc.vector.tensor_scalar_mul(rt[:, 0:NT], rt32[:, 0:NT], float(ALPHA / -GAMMA))
        nc.vector.memset(rt[:, NT:32], 1.0)
        for i in range(4):
            nc.vector.transpose(
                out=rrow[0:32, 32 * i:32 * i + 32],
                in_=rt[32 * i:32 * i + 32, 0:32],
            )
        # gather the 16 transposed rows into one q-major row on partition 0,
        # then broadcast it across all partitions in a single Pool op
        row0 = self.rzp.tile([1, S], BF16, tag="row0")
        nc.sync.dma_start(out=row0[0:1, 0:S], in_=rrow[0:NT, 0:128])
        nc.gpsimd.partition_broadcast(rbt[:], row0[0:1, 0:S], channels=128)

    def stage_clip(self, j, interleave_pv=False):
        """Ehat = E * rbt per strip and the dual-op clamp, emitted in 4
        group-prefix pieces so PV group g is gated only on its prefix.
        For the tail pair, PV groups (and early drains) interleave here."""
        nc = self.nc
        et, rbt = self.et[j], self.rbt[j]
        for g in range(4):
            kmax = 4 * g + 3
            for kk in range(4 * g, kmax + 1):
                lo, w = OFF_STRIP[kk], W_STRIP[kk]
                nc.vector.tensor_tensor(
                    et[:, lo:lo + w],
                    et[:, lo:lo + w],
                    rbt[:, 128 * kk:S],
                    mybir.AluOpType.mult,
                )
            clo = OFF_STRIP[4 * g]
            chi_ = OFF_STRIP[kmax] + W_STRIP[kmax]
            nc.vector.tensor_scalar(
                et[:, clo:chi_], et[:, clo:chi_], float(CHI), 1.0,
                mybir.AluOpType.min, mybir.AluOpType.max,
            )
            if interleave_pv:
                if g >= 2:
                    self.emit_drain(j, g - 2)  # free the psPV slot early
                self.emit_pv_group(j, g)

    def emit_pv_group(self, j, g):
        nc = self.nc
        et = self.et[j]
        if self.osb[j] is None:
            self.osb[j] = self.obp.tile([128, S], F32, tag="osb", name="osb")
        osb = self.osb[j]
        glo, ghi = 512 * g, 512 * (g + 1)
        kmax = 4 * g + 3
        ps = self.psPV.tile([128, 512], F32, tag="pv")
        self._pvps = getattr(self, "_pvps", {})
        self._pvps[(j, g)] = ps
        for kk in range(kmax + 1):
            qlo = max(glo, 128 * kk)
            src = OFF_STRIP[kk] + (qlo - 128 * kk)
            nc.tensor.matmul(
                ps[:, qlo - glo:512],
                lhsT=self._vblk(j, kk),
                rhs=et[:, src:src + (ghi - qlo)],
                start=(kk == 0), stop=(kk == kmax),
            )

    def emit_drain(self, j, g):
        done = getattr(self, "_drained", set())
        self._drained = done
        if (j, g) in done:
            return
        done.add((j, g))
        ps = self._pvps.pop((j, g))
        self.nc.vector.tensor_scalar_add(
            self.osb[j][:, 512 * g:512 * (g + 1)], ps[:, 0:512], 0.0
        )

    def emit_drains(self, j):
        for g in range(4):
            self.emit_drain(j, g)

    def finish_pv(self, j):
        self.nc.sync.dma_start(out=self.ot_d[j], in_=self.osb[j][:])


_NC_CACHE = None


def _get_program():
    global _NC_CACHE
    if _NC_CACHE is None:
        _NC_CACHE = build_core_program()
    return _NC_CACHE


def kernel(query_states, key_states, value_states, batch_size, q_length, kv_length):
    assert int(batch_size) == B and int(q_length) == S and int(kv_length) == S
    qf = np.asarray(query_states, dtype=np.float32).reshape(B, S, H, D)
    kf = np.asarray(key_states, dtype=np.float32).reshape(B, S, H, D)
    vf = np.asarray(value_states, dtype=np.float32).reshape(B, S, H, D)

    nc = _get_program()

    in_maps = []
    s_host = []  # per core: [NP, NT, D] causal-prefix sums of bf16 V
    for c in range(N_CORES):
        b = c // (N_CORES // B)
        h0 = NP * (c % (N_CORES // B))
        qb = qf[b, :, h0:h0 + NP, :].astype(ml_dtypes.bfloat16)  # [S, NP, D]
        kb = kf[b, :, h0:h0 + NP, :].astype(ml_dtypes.bfloat16)
        vb = vf[b, :, h0:h0 + NP, :].astype(ml_dtypes.bfloat16)
        in_maps.append(
            {
                "qt": np.ascontiguousarray(qb.transpose(1, 2, 0)),  # [NP,D,S]
                "kt": np.ascontiguousarray(kb.transpose(1, 2, 0)),
                "v": np.ascontiguousarray(vb.reshape(S, NP * D)),
            }
        )
        # S_T[d] = sum of V over k-tiles 0..T (f32 accumulation of bf16 V)
        vt32 = vb.astype(np.float32).reshape(NT, 128, NP, D)
        s_host.append(np.cumsum(vt32.sum(axis=1), axis=0).transpose(1, 0, 2))

    res = run_bass_kernel_spmd(nc, in_maps, list(range(N_CORES)))

    out = np.empty((B, S, H, D), dtype=np.float32)
    for c in range(N_CORES):
        b = c // (N_CORES // B)
        h0 = NP * (c % (N_CORES // B))
        ot = np.asarray(res.results[c]["ot"])  # [NP, D, S] = PV(t2)
        for jj in range(NP):
            pv = ot[jj].T.reshape(NT, 128, D)  # [T, q, D]
            pv = pv - s_host[c][jj][:, None, :]
            out[b, :, h0 + jj, :] = (-GAMMA) * pv.reshape(S, D)
    return out.reshape(B * S, H, D)


# revision 14
# speedup vs baseline: 1.0647x; 1.0156x over previous
"""Trainium2 Bass kernel for causal self-attention with clipped softmax.

Problem (hardcoded): B=2, S=2048, H=16, D=128, fp32 inputs.
    scores = (Q @ K^T) / sqrt(D), causal mask, p = softmax(scores)
    p = clip(1.06*p - 0.03, 0, 1)            # ZETA=1.03, GAMMA=-0.03
    out = p @ V

Sharding: 32 (batch, head) pairs -> 4 per core across 8 cores (tensor
parallel over heads + data parallel over batch). No cross-core comms.

Per-core device algorithm (transposed-scores layout, bf16 matmuls):
  - Q^T, K^T are staged host-side as [NP, D, S] so no on-device transposes;
    V is staged pair-interleaved [S, NP*D] for wide DMA descriptors.
  - scoresT[k, q] strips (causal only) are packed into one contiguous
    "stream" per pair so each exp activation covers a full PSUM tile.
  - E = exp(s/sqrt(D) + ln 1.06), diagonal masked by affine_select.
  - Z^T[q, tile] via tiny-output matmuls: E-block stationary, ones column
    moving -> PSUM accumulates the per-tile softmax denominators.
  - r[q] = (1.06/0.03)/Z' via one Reciprocal activation; transposed to a
    row with DVE 32x32 stream transposes; broadcast across partitions by
    GPSIMD partition_broadcast -> rbt[k, q] tile.
  - Ehat = E * rbt (one DVE tensor_tensor pass), then ONE dual-op
    tensor_scalar clamp: t2 = max(min(Ehat, 34.333), 1.0).
    Identity: clip(1.06p - 0.03, 0, 1) = 0.03*(t2 - 1), and masked
    positions (Ehat=0 -> t2=1) plus the -1 offset are together equal to
    the causal-prefix V sums, which the HOST subtracts after the fact:
      out[d, q in tile T] = 0.03*(PV(t2)[d, q] - S_T[d]),
      S_T[d] = sum of V[k, d] over all k-tiles <= T.
  - PV(t2) accumulated in PSUM over k-tiles, drained f32 to SBUF, DMA'd
    out as [NP, D, S] f32; host applies the S_T correction + 0.03 scale.
"""

import ml_dtypes
import numpy as np

import concourse.bass as bass
import concourse.mybir as mybir
import concourse.tile as tile
from concourse import bacc
from concourse.bass_utils import run_bass_kernel_spmd

B = 2
S = 2048
H = 16
D = 128
N_CORES = 8
NP = H * B // N_CORES  # (b,h) pairs per core = 4
NT = S // 128  # 128-col tiles along sequence = 16
INV_SQRT_D = 1.0 / np.sqrt(np.float64(D))
ZETA = 1.03
GAMMA = -0.03
ALPHA = ZETA - GAMMA  # 1.06
CHI = (1.0 - GAMMA) / (-GAMMA)  # 34.3333: upper clamp for Ehat
STREAM = S * NT - 64 * NT * (NT - 1)  # 17408 packed causal columns

F32 = mybir.dt.float32
BF16 = mybir.dt.bfloat16

# strip kk (k-tile) covers q in [128*kk, S); stream offset of each strip
W_STRIP = [S - 128 * kk for kk in range(NT)]
OFF_STRIP = [0] * NT
for _kk in range(1, NT):
    OFF_STRIP[_kk] = OFF_STRIP[_kk - 1] + W_STRIP[_kk - 1]

# exp/psum chunking of the packed stream: alternate 1536/1024 tiles
CHUNKS = []  # (stream_lo, stream_hi, which_pool)
_pos = 0
_tog = 0
while _pos < STREAM:
    cw = 1536 if _tog == 0 else 1024
    hi = min(_pos + cw, STREAM)
    CHUNKS.append((_pos, hi, _tog))
    _pos = hi
    _tog ^= 1


def _strip_of(pos):
    for kk in range(NT - 1, -1, -1):
        if pos >= OFF_STRIP[kk]:
            return kk
    raise AssertionError


def build_core_program():
    nc = bacc.Bacc(
        "TRN2", target_bir_lowering=False, debug=False, num_devices=N_CORES
    )

    qt_d = nc.dram_tensor("qt", [NP, D, S], BF16, kind="ExternalInput").ap()
    kt_d = nc.dram_tensor("kt", [NP, D, S], BF16, kind="ExternalInput").ap()
    v_d = nc.dram_tensor("v", [S, NP * D], BF16, kind="ExternalInput").ap()
    ot_d = nc.dram_tensor("ot", [NP, D, S], F32, kind="ExternalOutput").ap()

    with tile.TileContext(nc) as tc:
        Builder(tc, qt_d, kt_d, v_d, ot_d).build()

    nc.compile()
    return nc


class Builder:
    def __init__(self, tc, qt_d, kt_d, v_d, ot_d):
        self.tc = tc
        self.nc = tc.nc
        self.qt_d, self.kt_d, self.v_d, self.ot_d = qt_d, kt_d, v_d, ot_d
        self.qt = [None] * NP
        self.kt = [None] * NP
        self.et = [None] * NP
        self.rbt = [None] * NP
        self.psz = [None] * NP
        self.osb = [None] * NP

    def build(self):
        nc = self.nc
        with (
            self.tc.tile_pool(name="const", bufs=1) as constp,
            self.tc.tile_pool(name="vt", bufs=1) as vtp,
            self.tc.tile_pool(name="qk", bufs=2) as qkp,
            self.tc.tile_pool(name="et", bufs=2) as etp,
            self.tc.tile_pool(name="rz", bufs=2) as rzp,
            self.tc.tile_pool(name="rb", bufs=2) as rbp,
            self.tc.tile_pool(name="ob", bufs=2) as obp,
            self.tc.tile_pool(name="psA", bufs=1, space="PSUM") as psA,
            self.tc.tile_pool(name="psB", bufs=1, space="PSUM") as psB,
            self.tc.tile_pool(name="psPV", bufs=2, space="PSUM") as psPV,
            self.tc.tile_pool(name="psZ", bufs=1, space="PSUM") as psZ,
        ):
            self.qkp, self.etp, self.rzp, self.rbp, self.obp = (
                qkp, etp, rzp, rbp, obp,
            )
            self.psA, self.psB, self.psPV, self.psZ = psA, psB, psPV, psZ
            self.vtp = vtp

            self.ones_blk = constp.tile([128, 512], BF16)
            nc.vector.memset(self.ones_blk[:], 1.0)
            self.bias_ln = constp.tile([128, 1], F32)
            nc.vector.memset(self.bias_ln[:], float(np.log(ALPHA)))

            # PE p-state warmup: keep the PE busy from t=0 so the clock is
            # fully ramped (>3us continuous) when real matmuls arrive.
            wps = self.psA.tile([128, 1536], F32, tag="schunk")
            for i in range(6):
                nc.tensor.matmul(
                    wps[:, (i % 3) * 512:(i % 3) * 512 + 512],
                    lhsT=self.ones_blk[:, 0:128],
                    rhs=self.ones_blk[:],
                    start=True, stop=True,
                )

            self.stage_in(0)
            self.stage_in(1)
            self.stage_in_v()
            self.stage_qk(0)
            self.stage_zfin(0)
            self.stage_clip(0)
            self.stage_in(2)
            self.stage_qk(1, pv_pair=0)
            self.stage_zfin(1)
            self.emit_drains(0)
            self.finish_pv(0)
            self.stage_clip(1)
            self.stage_in(3)
            self.stage_qk(2, pv_pair=1)
            self.stage_zfin(2)
            self.emit_drains(1)
            self.finish_pv(1)
            self.stage_clip(2)
            self.stage_qk(3, pv_pair=2)
            self.stage_zfin(3)
            self.emit_drains(2)
            self.finish_pv(2)
            self.stage_clip(3, interleave_pv=True)
            self.emit_drains(3)
            self.finish_pv(3)

    def stage_in_v(self):
        # one wide DMA for all pairs' V: [S, NP*D] -> [128, (T, NP*D)]
        self.vt = self.vtp.tile([128, NT * NP * D], BF16)
        self.nc.sync.dma_start(
            out=self.vt[:].rearrange("p (t x) -> p t x", x=NP * D),
            in_=self.v_d.rearrange("(t p) x -> p t x", p=128),
        )

    def stage_in(self, j):
        nc = self.nc
        qt = self.qkp.tile([128, S], BF16, tag="qt")
        kt = self.qkp.tile([128, S], BF16, tag="kt")
        nc.sync.dma_start(out=kt[:], in_=self.kt_d[j])
        nc.sync.dma_start(out=qt[:], in_=self.qt_d[j])
        self.qt[j], self.kt[j] = qt, kt

    def _vblk(self, j, kk):
        off = (kk * NP + j) * D
        return self.vt[:, off:off + D]

    def stage_qk(self, j, pv_pair=None):
        """QK^T strips packed into alternating PSUM chunks + exp + inline
        Pool diag masks; the mini-Z matmuls go AFTER all chunks so they
        never block the PE queue mid-stream. Optionally interleaves the
        previous pair's clip/PV steps between chunks as PE filler."""
        nc = self.nc
        qt, kt = self.qt[j], self.kt[j]
        et = self.etp.tile([128, STREAM], BF16, tag="et")
        psz = self.psZ.tile([128, NT], F32, tag="zt")
        self.et[j] = et
        self.psz[j] = psz

        # fillers: previous pair's PV groups at LATE chunk slots, by which
        # time its clip prefixes are done on DVE so the PE queue never
        # blocks on the clip (drains are emitted later, after zfin(j))
        fillers = {}
        if pv_pair is not None:
            fillers = {
                10: lambda: self.emit_pv_group(pv_pair, 0),
                11: lambda: self.emit_pv_group(pv_pair, 1),
                12: lambda: self.emit_pv_group(pv_pair, 2),
                13: lambda: self.emit_pv_group(pv_pair, 3),
            }

        for ci, (lo, hi, tog) in enumerate(CHUNKS):
            pool = self.psA if tog == 0 else self.psB
            width = 1536 if tog == 0 else 1024
            ps = pool.tile([128, width], F32, tag="schunk")
            # matmul pieces: split at strip boundaries and 512-grid of tile
            pos = lo
            while pos < hi:
                kk = _strip_of(pos)
                strip_end = OFF_STRIP[kk] + W_STRIP[kk]
                seg_end = min(hi, strip_end, lo + ((pos - lo) // 512 + 1) * 512)
                qa = 128 * kk + (pos - OFF_STRIP[kk])
                qb = qa + (seg_end - pos)
                nc.tensor.matmul(
                    ps[:, pos - lo:seg_end - lo],
                    lhsT=kt[:, 128 * kk:128 * kk + 128],
                    rhs=qt[:, qa:qb],
                    start=True, stop=True,
                )
                pos = seg_end
            nc.scalar.activation(
                et[:, lo:hi],
                ps[:, 0:hi - lo],
                mybir.ActivationFunctionType.Exp,
                scale=float(INV_SQRT_D),
                bias=self.bias_ln[:],
            )
            # Pool-only diagonal masks as their chunk lands
            for kk in range(NT):
                dlo = OFF_STRIP[kk]
                if lo <= dlo and dlo + 128 <= hi:
                    nc.gpsimd.affine_select(
                        out=et[:, dlo:dlo + 128],
                        in_=et[:, dlo:dlo + 128],
                        compare_op=mybir.AluOpType.is_ge,
                        fill=0.0,
                        base=0,
                        pattern=[[1, 128]],
                        channel_multiplier=-1,
                    )
            if ci in fillers:
                fillers.pop(ci)()
        # mini-Z: Z^T column T accumulates E over strips 0..T at q-tile T
        for T in range(NT):
            for k2 in range(T + 1):
                blk = OFF_STRIP[k2] + 128 * (T - k2)
                nc.tensor.matmul(
                    psz[:, T:T + 1],
                    lhsT=et[:, blk:blk + 128],
                    rhs=self.ones_blk[:, 0:1],
                    start=(k2 == 0), stop=(k2 == T),
                )
        for ci in sorted(fillers):
            fillers.pop(ci)()

    def stage_zfin(self, j):
        """Reciprocal + transpose to row + partition broadcast -> rbt."""
        nc = self.nc
        rt32 = self.rzp.tile([128, 16], F32, tag="rt32")
        rt = self.rzp.tile([128, 32], BF16, tag="rt")
        rrow = self.rzp.tile([32, 128], BF16, tag="rrow")
        rbt = self.rbp.tile([128, S], BF16, tag="rbt")
        self.rbt[j] = rbt
        # r = (ALPHA/0.03)/Z'  (constant folded into the bf16 cast below)
        nc.vector.reciprocal(rt32[:, 0:NT], self.psz[j][:, 0:NT])
        nc.vector.tensor_scalar_mul(rt[:, 0:NT], rt32[:, 0:NT], float(ALPHA / -GAMMA))
        nc.vector.memset(rt[:, NT:32], 1.0)
        for i in range(4):
            nc.vector.transpose(
                out=rrow[0:32, 32 * i:32 * i + 32],
                in_=rt[32 * i:32 * i + 32, 0:32],
            )
        # gather the 16 transposed rows into one q-major row on partition 0,
        # then broadcast it across all partitions in a single Pool op
        row0 = self.rzp.tile([1, S], BF16, tag="row0")
        nc.sync.dma_start(out=row0[0:1, 0:S], in_=rrow[0:NT, 0:128])
        nc.gpsimd.partition_broadcast(rbt[:], row0[0:1, 0:S], channels=128)

    def stage_clip(self, j, interleave_pv=False):
        """Ehat = E * rbt per strip and the dual-op clamp, emitted in 4
        group-prefix pieces so PV group g is gated only on its prefix.
        For the tail pair, PV groups (and early drains) interleave here."""
        nc = self.nc
        et, rbt = self.et[j], self.rbt[j]
        for g in range(4):
            kmax = 4 * g + 3
            for kk in range(4 * g, kmax + 1):
                lo, w = OFF_STRIP[kk], W_STRIP[kk]
                nc.vector.tensor_tensor(
                    et[:, lo:lo + w],
                    et[:, lo:lo + w],
                    rbt[:, 128 * kk:S],
                    mybir.AluOpType.mult,
                )
            clo = OFF_STRIP[4 * g]
            chi_ = OFF_STRIP[kmax] + W_STRIP[kmax]
            nc.vector.tensor_scalar(
                et[:, clo:chi_], et[:, clo:chi_], float(CHI), 1.0,
                mybir.AluOpType.min, mybir.AluOpType.max,
            )
            if interleave_pv:
                if g >= 2:
                    self.emit_drain(j, g - 2)  # free the psPV slot early
                self.emit_pv_group(j, g)

    def emit_pv_group(self, j, g):
        nc = self.nc
        et = self.et[j]
        if self.osb[j] is None:
            self.osb[j] = self.obp.tile([128, S], F32, tag="osb", name="osb")
        osb = self.osb[j]
        glo, ghi = 512 * g, 512 * (g + 1)
        kmax = 4 * g + 3
        ps = self.psPV.tile([128, 512], F32, tag="pv")
        self._pvps = getattr(self, "_pvps", {})
        self._pvps[(j, g)] = ps
        for kk in range(kmax + 1):
            qlo = max(glo, 128 * kk)
            src = OFF_STRIP[kk] + (qlo - 128 * kk)
            nc.tensor.matmul(
                ps[:, qlo - glo:512],
                lhsT=self._vblk(j, kk),
                rhs=et[:, src:src + (ghi - qlo)],
                start=(kk == 0), stop=(kk == kmax),
            )

    def emit_drain(self, j, g):
        done = getattr(self, "_drained", set())
        self._drained = done
        if (j, g) in done:
            return
        done.add((j, g))
        ps = self._pvps.pop((j, g))
        self.nc.vector.tensor_scalar_add(
            self.osb[j][:, 512 * g:512 * (g + 1)], ps[:, 0:512], 0.0
        )

    def emit_drains(self, j):
        for g in range(4):
            self.emit_drain(j, g)

    def finish_pv(self, j):
        self.nc.sync.dma_start(out=self.ot_d[j], in_=self.osb[j][:])


_NC_CACHE = None


def _get_program():
    global _NC_CACHE
    if _NC_CACHE is None:
        _NC_CACHE = build_core_program()
    return _NC_CACHE


def kernel(query_states, key_states, value_states, batch_size, q_length, kv_length):
    assert int(batch_size) == B and int(q_length) == S and int(kv_length) == S
    qf = np.asarray(query_states, dtype=np.float32).reshape(B, S, H, D)
    kf = np.asarray(key_states, dtype=np.float32).reshape(B, S, H, D)
    vf = np.asarray(value_states, dtype=np.float32).reshape(B, S, H, D)

    nc = _get_program()

    in_maps = []
    s_host = []  # per core: [NP, NT, D] causal-prefix sums of bf16 V
    for c in range(N_CORES):
        b = c // (N_CORES // B)
        h0 = NP * (c % (N_CORES // B))
        qb = qf[b, :, h0:h0 + NP, :].astype(ml_dtypes.bfloat16)  # [S, NP, D]
        kb = kf[b, :, h0:h0 + NP, :].astype(ml_dtypes.bfloat16)
        vb = vf[b, :, h0:h0 + NP, :].astype(ml_dtypes.bfloat16)
        in_maps.append(
            {
                "qt": np.ascontiguousarray(qb.transpose(1, 2, 0)),  # [NP,D,S]
                "kt": np.ascontiguousarray(kb.transpose(1, 2, 0)),
                "v": np.ascontiguousarray(vb.reshape(S, NP * D)),
            }
        )
        # S_T[d] = sum of V over k-tiles 0..T (f32 accumulation of bf16 V)
        vt32 = vb.astype(np.float32).reshape(NT, 128, NP, D)
        s_host.append(np.cumsum(vt32.sum(axis=1), axis=0).transpose(1, 0, 2))

    res = run_bass_kernel_spmd(nc, in_maps, list(range(N_CORES)))

    out = np.empty((B, S, H, D), dtype=np.float32)
    for c in range(N_CORES):
        b = c // (N_CORES // B)
        h0 = NP * (c % (N_CORES // B))
        ot = np.asarray(res.results[c]["ot"])  # [NP, D, S] = PV(t2)
        for jj in range(NP):
            pv = ot[jj].T.reshape(NT, 128, D)  # [T, q, D]
            pv = pv - s_host[c][jj][:, None, :]
            out[b, :, h0 + jj, :] = (-GAMMA) * pv.reshape(S, D)
    return out.reshape(B * S, H, D)


# revision 17
# speedup vs baseline: 1.1876x; 1.1155x over previous
"""Trainium2 Bass kernel for causal self-attention with clipped softmax.

Problem (hardcoded): B=2, S=2048, H=16, D=128, fp32 inputs.
    scores = (Q @ K^T) / sqrt(D), causal mask, p = softmax(scores)
    p = clip(1.06*p - 0.03, 0, 1)            # ZETA=1.03, GAMMA=-0.03
    out = p @ V

Sharding: 32 (batch, head) pairs -> 4 per core across 8 cores (tensor
parallel over heads + data parallel over batch). No cross-core comms.

Per-core device algorithm (transposed-scores layout, bf16 matmuls):
  - Q^T, K^T are staged host-side as [NP, D, S] so no on-device transposes;
    V is staged pair-interleaved [S, NP*D] for wide DMA descriptors.
  - scoresT[k, q] strips (causal only) are packed into one contiguous
    "stream" per pair so each exp activation covers a full PSUM tile.
  - E = exp(s/sqrt(D) + ln 1.06), diagonal masked by affine_select.
  - Z^T[q, tile] via tiny-output matmuls: E-block stationary, ones column
    moving -> PSUM accumulates the per-tile softmax denominators.
  - r[q] = (1.06/0.03)/Z' via one Reciprocal activation; transposed to a
    row with DVE 32x32 stream transposes; broadcast across partitions by
    GPSIMD partition_broadcast -> rbt[k, q] tile.
  - Ehat = E * rbt (one DVE tensor_tensor pass), then ONE dual-op
    tensor_scalar clamp: t2 = max(min(Ehat, 34.333), 1.0).
    Identity: clip(1.06p - 0.03, 0, 1) = 0.03*(t2 - 1), and masked
    positions (Ehat=0 -> t2=1) plus the -1 offset are together equal to
    the causal-prefix V sums, which the HOST subtracts after the fact:
      out[d, q in tile T] = 0.03*(PV(t2)[d, q] - S_T[d]),
      S_T[d] = sum of V[k, d] over all k-tiles <= T.
  - PV(t2) accumulated in PSUM over k-tiles, drained f32 to SBUF, DMA'd
    out as [NP, D, S] f32; host applies the S_T correction + 0.03 scale.
"""

import ml_dtypes
import numpy as np

import concourse.bass as bass
import concourse.mybir as mybir
import concourse.tile as tile
from concourse import bacc
from concourse.bass_utils import run_bass_kernel_spmd

B = 2
S = 2048
H = 16
D = 128
N_CORES = 8
NP = H * B // N_CORES  # (b,h) pairs per core = 4
NT = S // 128  # 128-col tiles along sequence = 16
INV_SQRT_D = 1.0 / np.sqrt(np.float64(D))
ZETA = 1.03
GAMMA = -0.03
ALPHA = ZETA - GAMMA  # 1.06
CHI = (1.0 - GAMMA) / (-GAMMA)  # 34.3333: upper clamp for Ehat
STREAM = S * NT - 64 * NT * (NT - 1)  # 17408 packed causal columns

F32 = mybir.dt.float32
BF16 = mybir.dt.bfloat16

# strip kk (k-tile) covers q in [128*kk, S); stream offset of each strip
W_STRIP = [S - 128 * kk for kk in range(NT)]
OFF_STRIP = [0] * NT
for _kk in range(1, NT):
    OFF_STRIP[_kk] = OFF_STRIP[_kk - 1] + W_STRIP[_kk - 1]

# exp/psum chunking of the packed stream: alternate 1536/1024 tiles
CHUNKS = []  # (stream_lo, stream_hi, which_pool)
_pos = 0
_tog = 0
while _pos < STREAM:
    hi = min(_pos + 1024, STREAM)
    CHUNKS.append((_pos, hi, _tog))
    _pos = hi
    _tog ^= 1


def _strip_of(pos):
    for kk in range(NT - 1, -1, -1):
        if pos >= OFF_STRIP[kk]:
            return kk
    raise AssertionError


def build_core_program():
    nc = bacc.Bacc(
        "TRN2", target_bir_lowering=False, debug=False, num_devices=N_CORES
    )

    qt_d = nc.dram_tensor("qt", [NP, D, S], BF16, kind="ExternalInput").ap()
    kt_d = nc.dram_tensor("kt", [NP, D, S], BF16, kind="ExternalInput").ap()
    v_d = nc.dram_tensor("v", [S, NP * D], BF16, kind="ExternalInput").ap()
    ot_d = nc.dram_tensor("ot", [NP, D, S], F32, kind="ExternalOutput").ap()

    with tile.TileContext(nc) as tc:
        Builder(tc, qt_d, kt_d, v_d, ot_d).build()

    nc.compile()
    return nc


class Builder:
    def __init__(self, tc, qt_d, kt_d, v_d, ot_d):
        self.tc = tc
        self.nc = tc.nc
        self.qt_d, self.kt_d, self.v_d, self.ot_d = qt_d, kt_d, v_d, ot_d
        self.qt = [None] * NP
        self.kt = [None] * NP
        self.et = [None] * NP
        self.rbt = [None] * NP
        self.psz = [None] * NP
        self.osb = [None] * NP

    def build(self):
        nc = self.nc
        with (
            self.tc.tile_pool(name="const", bufs=1) as constp,
            self.tc.tile_pool(name="vt", bufs=1) as vtp,
            self.tc.tile_pool(name="qk", bufs=2) as qkp,
            self.tc.tile_pool(name="et", bufs=2) as etp,
            self.tc.tile_pool(name="rz", bufs=2) as rzp,
            self.tc.tile_pool(name="rb", bufs=2) as rbp,
            self.tc.tile_pool(name="ob", bufs=2) as obp,
            self.tc.tile_pool(name="psA", bufs=1, space="PSUM") as psA,
            self.tc.tile_pool(name="psB", bufs=1, space="PSUM") as psB,
            self.tc.tile_pool(name="psPV", bufs=3, space="PSUM") as psPV,
            self.tc.tile_pool(name="psZ", bufs=1, space="PSUM") as psZ,
        ):
            self.qkp, self.etp, self.rzp, self.rbp, self.obp = (
                qkp, etp, rzp, rbp, obp,
            )
            self.psA, self.psB, self.psPV, self.psZ = psA, psB, psPV, psZ
            self.vtp = vtp

            self.ones_blk = constp.tile([128, 512], BF16)
            nc.vector.memset(self.ones_blk[:], 1.0)
            self.bias_ln = constp.tile([128, 1], F32)
            nc.vector.memset(self.bias_ln[:], float(np.log(ALPHA)))

            # PE p-state warmup: keep the PE busy from t=0 so the clock is
            # fully ramped (>3us continuous) when real matmuls arrive.
            wps = self.psA.tile([128, 1024], F32, tag="schunk")
            for i in range(6):
                nc.tensor.matmul(
                    wps[:, (i % 2) * 512:(i % 2) * 512 + 512],
                    lhsT=self.ones_blk[:, 0:128],
                    rhs=self.ones_blk[:],
                    start=True, stop=True,
                )

            self.stage_in(0)
            self.stage_in(1)
            self.stage_in_v()
            self.stage_qk(0)
            self.stage_zfin(0)
            self.stage_clip(0)
            self.stage_in(2)
            self.stage_qk(1, pv_pair=0)
            self.stage_zfin(1)
            self.stage_clip(1, drain_pair=0)
            self.stage_in(3)
            self.stage_qk(2, pv_pair=1)
            self.stage_zfin(2)
            self.stage_clip(2, drain_pair=1)
            self.stage_qk(3, pv_pair=2)
            self.stage_zfin(3)
            self.stage_clip(3, drain_pair=2, interleave_pv=True)
            self.emit_drains(3)
            self.finish_pv(3)

    def stage_in_v(self):
        # one wide DMA for all pairs' V: [S, NP*D] -> [128, (T, NP*D)]
        self.vt = self.vtp.tile([128, NT * NP * D], BF16)
        self.nc.sync.dma_start(
            out=self.vt[:].rearrange("p (t x) -> p t x", x=NP * D),
            in_=self.v_d.rearrange("(t p) x -> p t x", p=128),
        )

    def stage_in(self, j):
        nc = self.nc
        qt = self.qkp.tile([128, S], BF16, tag="qt")
        kt = self.qkp.tile([128, S], BF16, tag="kt")
        nc.sync.dma_start(out=kt[:], in_=self.kt_d[j])
        nc.sync.dma_start(out=qt[:], in_=self.qt_d[j])
        self.qt[j], self.kt[j] = qt, kt

    def _vblk(self, j, kk):
        off = (kk * NP + j) * D
        return self.vt[:, off:off + D]

    def stage_qk(self, j, pv_pair=None):
        """QK^T strips packed into alternating PSUM chunks + exp + inline
        Pool diag masks; the mini-Z matmuls go AFTER all chunks so they
        never block the PE queue mid-stream. Optionally interleaves the
        previous pair's clip/PV steps between chunks as PE filler."""
        nc = self.nc
        qt, kt = self.qt[j], self.kt[j]
        et = self.etp.tile([128, STREAM], BF16, tag="et")
        psz = self.psZ.tile([128, NT], F32, tag="zt")
        self.et[j] = et
        self.psz[j] = psz

        # fillers: previous pair's PV groups at LATE chunk slots, by which
        # time its clip prefixes are done on DVE so the PE queue never
        # blocks on the clip (drains are emitted later, after zfin(j))
        fillers = {}
        if pv_pair is not None:
            for slot, g in zip((9, 11, 13), range(3)):
                fillers[slot] = (
                    lambda g=g: self.emit_pv_group(pv_pair, g)
                )

        for ci, (lo, hi, tog) in enumerate(CHUNKS):
            pool = self.psA if tog == 0 else self.psB
            ps = pool.tile([128, 1024], F32, tag="schunk")
            # matmul pieces: split at strip boundaries and 512-grid of tile
            pos = lo
            while pos < hi:
                kk = _strip_of(pos)
                strip_end = OFF_STRIP[kk] + W_STRIP[kk]
                seg_end = min(hi, strip_end, lo + ((pos - lo) // 512 + 1) * 512)
                qa = 128 * kk + (pos - OFF_STRIP[kk])
                qb = qa + (seg_end - pos)
                nc.tensor.matmul(
                    ps[:, pos - lo:seg_end - lo],
                    lhsT=kt[:, 128 * kk:128 * kk + 128],
                    rhs=qt[:, qa:qb],
                    start=True, stop=True,
                )
                pos = seg_end
            nc.scalar.activation(
                et[:, lo:hi],
                ps[:, 0:hi - lo],
                mybir.ActivationFunctionType.Exp,
                scale=float(INV_SQRT_D),
                bias=self.bias_ln[:],
            )
            # Pool-only diagonal masks as their chunk lands
            for kk in range(NT):
                dlo = OFF_STRIP[kk]
                if lo <= dlo and dlo + 128 <= hi:
                    nc.gpsimd.affine_select(
                        out=et[:, dlo:dlo + 128],
                        in_=et[:, dlo:dlo + 128],
                        compare_op=mybir.AluOpType.is_ge,
                        fill=0.0,
                        base=0,
                        pattern=[[1, 128]],
                        channel_multiplier=-1,
                    )
            if ci in fillers:
                fillers.pop(ci)()
        # mini-Z: Z^T column T accumulates E over strips 0..T at q-tile T
        for T in range(NT):
            for k2 in range(T + 1):
                blk = OFF_STRIP[k2] + 128 * (T - k2)
                nc.tensor.matmul(
                    psz[:, T:T + 1],
                    lhsT=et[:, blk:blk + 128],
                    rhs=self.ones_blk[:, 0:1],
                    start=(k2 == 0), stop=(k2 == T),
                )
        for ci in sorted(fillers):
            fillers.pop(ci)()
        if pv_pair is not None:
            self.emit_drain(pv_pair, 0)
            self.emit_pv_group(pv_pair, 3)

    def stage_zfin(self, j):
        """Reciprocal + transpose to row + partition broadcast -> rbt."""
        nc = self.nc
        rt32 = self.rzp.tile([128, 16], F32, tag="rt32")
        rt = self.rzp.tile([128, 32], BF16, tag="rt")
        rrow = self.rzp.tile([32, 128], BF16, tag="rrow")
        rbt = self.rbp.tile([128, S], BF16, tag="rbt")
        self.rbt[j] = rbt
        # r = (ALPHA/0.03)/Z'  (constant folded into the bf16 cast below)
        nc.vector.reciprocal(rt32[:, 0:NT], self.psz[j][:, 0:NT])
        nc.vector.tensor_scalar_mul(rt[:, 0:NT], rt32[:, 0:NT], float(ALPHA / -GAMMA))
        nc.vector.memset(rt[:, NT:32], 1.0)
        for i in range(4):
            nc.vector.transpose(
                out=rrow[0:32, 32 * i:32 * i + 32],
                in_=rt[32 * i:32 * i + 32, 0:32],
            )
        # gather the 16 transposed rows into one q-major row on partition 0,
        # then broadcast it across all partitions in a single Pool op
        row0 = self.rzp.tile([1, S], BF16, tag="row0")
        nc.sync.dma_start(out=row0[0:1, 0:S], in_=rrow[0:NT, 0:128])
        nc.gpsimd.partition_broadcast(rbt[:], row0[0:1, 0:S], channels=128)

    def stage_clip(self, j, drain_pair=None, interleave_pv=False):
        """Ehat = E * rbt per strip and the dual-op clamp, in 4 prefix
        pieces. Runs on DVE concurrently with qk(j+1); the previous
        pair's remaining PSUM drains execute up front (their PV groups
        finished last window)."""
        nc = self.nc
        et, rbt = self.et[j], self.rbt[j]
        if drain_pair is not None:
            for g in range(1, 4):
                self.emit_drain(drain_pair, g)
            self.finish_pv(drain_pair)
        for g in range(4):
            kmax = 4 * g + 3
            for kk in range(4 * g, kmax + 1):
                lo, w = OFF_STRIP[kk], W_STRIP[kk]
                nc.vector.tensor_tensor(
                    et[:, lo:lo + w],
                    et[:, lo:lo + w],
                    rbt[:, 128 * kk:S],
                    mybir.AluOpType.mult,
                )
            clo = OFF_STRIP[4 * g]
            chi_ = OFF_STRIP[kmax] + W_STRIP[kmax]
            nc.vector.tensor_scalar(
                et[:, clo:chi_], et[:, clo:chi_], float(CHI), 1.0,
                mybir.AluOpType.min, mybir.AluOpType.max,
            )
            if interleave_pv:
                if g == 3:
                    self.emit_drain(j, 0)
                self.emit_pv_group(j, g)

    def emit_pv_group(self, j, g):
        nc = self.nc
        et = self.et[j]
        if self.osb[j] is None:
            self.osb[j] = self.obp.tile([128, S], F32, tag="osb", name="osb")
        glo, ghi = 512 * g, 512 * (g + 1)
        kmax = 4 * g + 3
        ps = self.psPV.tile([128, 512], F32, tag="pv")
        self._pvps = getattr(self, "_pvps", {})
        self._pvps[(j, g)] = ps
        for kk in range(kmax + 1):
            qlo = max(glo, 128 * kk)
            src = OFF_STRIP[kk] + (qlo - 128 * kk)
            nc.tensor.matmul(
                ps[:, qlo - glo:512],
                lhsT=self._vblk(j, kk),
                rhs=et[:, src:src + (ghi - qlo)],
                start=(kk == 0), stop=(kk == kmax),
            )

    def emit_drain(self, j, g):
        done = getattr(self, "_drained", set())
        self._drained = done
        if (j, g) in done or (j, g) not in getattr(self, "_pvps", {}):
            return
        done.add((j, g))
        ps = self._pvps.pop((j, g))
        self.nc.vector.tensor_scalar_add(
            self.osb[j][:, 512 * g:512 * (g + 1)], ps[:, 0:512], 0.0
        )

    def emit_drains(self, j):
        for g in range(4):
            self.emit_drain(j, g)

    def finish_pv(self, j):
        self.nc.sync.dma_start(out=self.ot_d[j], in_=self.osb[j][:])


_NC_CACHE = None


def _get_program():
    global _NC_CACHE
    if _NC_CACHE is None:
        _NC_CACHE = build_core_program()
    return _NC_CACHE


def kernel(query_states, key_states, value_states, batch_size, q_length, kv_length):
    assert int(batch_size) == B and int(q_length) == S and int(kv_length) == S
    qf = np.asarray(query_states, dtype=np.float32).reshape(B, S, H, D)
    kf = np.asarray(key_states, dtype=np.float32).reshape(B, S, H, D)
    vf = np.asarray(value_states, dtype=np.float32).reshape(B, S, H, D)

    nc = _get_program()

    in_maps = []
    s_host = []  # per core: [NP, NT, D] causal-prefix sums of bf16 V
    for c in range(N_CORES):
        b = c // (N_CORES // B)
        h0 = NP * (c % (N_CORES // B))
        qb = qf[b, :, h0:h0 + NP, :].astype(ml_dtypes.bfloat16)  # [S, NP, D]
        kb = kf[b, :, h0:h0 + NP, :].astype(ml_dtypes.bfloat16)
        vb = vf[b, :, h0:h0 + NP, :].astype(ml_dtypes.bfloat16)
        in_maps.append(
            {
                "qt": np.ascontiguousarray(qb.transpose(1, 2, 0)),  # [NP,D,S]
                "kt": np.ascontiguousarray(kb.transpose(1, 2, 0)),
                "v": np.ascontiguousarray(vb.reshape(S, NP * D)),
            }
        )
        # S_T[d] = sum of V over k-tiles 0..T (f32 accumulation of bf16 V)
        vt32 = vb.astype(np.float32).reshape(NT, 128, NP, D)
        s_host.append(np.cumsum(vt32.sum(axis=1), axis=0).transpose(1, 0, 2))

    res = run_bass_kernel_spmd(nc, in_maps, list(range(N_CORES)))

    out = np.empty((B, S, H, D), dtype=np.float32)
    for c in range(N_CORES):
        b = c // (N_CORES // B)
        h0 = NP * (c % (N_CORES // B))
        ot = np.asarray(res.results[c]["ot"])  # [NP, D, S] = PV(t2)
        for jj in range(NP):
            pv = ot[jj].T.reshape(NT, 128, D)  # [T, q, D]
            pv = pv - s_host[c][jj][:, None, :]
            out[b, :, h0 + jj, :] = (-GAMMA) * pv.reshape(S, D)
    return out.reshape(B * S, H, D)


# revision 19
# speedup vs baseline: 1.2104x; 1.0192x over previous
"""Trainium2 Bass kernel for causal self-attention with clipped softmax.

Problem (hardcoded): B=2, S=2048, H=16, D=128, fp32 inputs.
    scores = (Q @ K^T) / sqrt(D), causal mask, p = softmax(scores)
    p = clip(1.06*p - 0.03, 0, 1)            # ZETA=1.03, GAMMA=-0.03
    out = p @ V

Sharding: 32 (batch, head) pairs -> 4 per core across 8 cores (tensor
parallel over heads + data parallel over batch). No cross-core comms.

Per-core device algorithm (transposed-scores layout, bf16 matmuls):
  - Q^T, K^T are staged host-side as [NP, D, S] so no on-device transposes;
    V is staged pair-interleaved [S, NP*D] for wide DMA descriptors.
  - scoresT[k, q] strips (causal only) are packed into one contiguous
    "stream" per pair so each exp activation covers a full PSUM tile.
  - E = exp(s/sqrt(D) + ln 1.06), diagonal masked by affine_select.
  - Z^T[q, tile] via tiny-output matmuls: E-block stationary, ones column
    moving -> PSUM accumulates the per-tile softmax denominators.
  - r[q] = (1.06/0.03)/Z' via one Reciprocal activation; transposed to a
    row with DVE 32x32 stream transposes; broadcast across partitions by
    GPSIMD partition_broadcast -> rbt[k, q] tile.
  - Ehat = E * rbt (one DVE tensor_tensor pass), then ONE dual-op
    tensor_scalar clamp: t2 = max(min(Ehat, 34.333), 1.0).
    Identity: clip(1.06p - 0.03, 0, 1) = 0.03*(t2 - 1), and masked
    positions (Ehat=0 -> t2=1) plus the -1 offset are together equal to
    the causal-prefix V sums, which the HOST subtracts after the fact:
      out[d, q in tile T] = 0.03*(PV(t2)[d, q] - S_T[d]),
      S_T[d] = sum of V[k, d] over all k-tiles <= T.
  - PV(t2) accumulated in PSUM over k-tiles, drained f32 to SBUF, DMA'd
    out as [NP, D, S] f32; host applies the S_T correction + 0.03 scale.
"""

import ml_dtypes
import numpy as np

import concourse.bass as bass
import concourse.mybir as mybir
import concourse.tile as tile
from concourse import bacc
from concourse.bass_utils import run_bass_kernel_spmd

B = 2
S = 2048
H = 16
D = 128
N_CORES = 8
NP = H * B // N_CORES  # (b,h) pairs per core = 4
NT = S // 128  # 128-col tiles along sequence = 16
INV_SQRT_D = 1.0 / np.sqrt(np.float64(D))
ZETA = 1.03
GAMMA = -0.03
ALPHA = ZETA - GAMMA  # 1.06
CHI = (1.0 - GAMMA) / (-GAMMA)  # 34.3333: upper clamp for Ehat
STREAM = S * NT - 64 * NT * (NT - 1)  # 17408 packed causal columns

F32 = mybir.dt.float32
BF16 = mybir.dt.bfloat16

# strip kk (k-tile) covers q in [128*kk, S); stream offset of each strip
W_STRIP = [S - 128 * kk for kk in range(NT)]
OFF_STRIP = [0] * NT
for _kk in range(1, NT):
    OFF_STRIP[_kk] = OFF_STRIP[_kk - 1] + W_STRIP[_kk - 1]

# exp/psum chunking of the packed stream: alternate 1536/1024 tiles
CHUNKS = []  # (stream_lo, stream_hi, which_pool)
_pos = 0
_tog = 0
while _pos < STREAM:
    hi = min(_pos + 1024, STREAM)
    CHUNKS.append((_pos, hi, _tog))
    _pos = hi
    _tog ^= 1


def _strip_of(pos):
    for kk in range(NT - 1, -1, -1):
        if pos >= OFF_STRIP[kk]:
            return kk
    raise AssertionError


def build_core_program():
    nc = bacc.Bacc(
        "TRN2", target_bir_lowering=False, debug=False, num_devices=N_CORES
    )

    qt_d = nc.dram_tensor("qt", [NP, D, S], BF16, kind="ExternalInput").ap()
    kt_d = nc.dram_tensor("kt", [NP, D, S], BF16, kind="ExternalInput").ap()
    v_d = nc.dram_tensor("v", [S, NP * D], BF16, kind="ExternalInput").ap()
    ot_d = nc.dram_tensor("ot", [NP, D, S], F32, kind="ExternalOutput").ap()

    with tile.TileContext(nc) as tc:
        Builder(tc, qt_d, kt_d, v_d, ot_d).build()

    nc.compile()
    return nc


class Builder:
    def __init__(self, tc, qt_d, kt_d, v_d, ot_d):
        self.tc = tc
        self.nc = tc.nc
        self.qt_d, self.kt_d, self.v_d, self.ot_d = qt_d, kt_d, v_d, ot_d
        self.qt = [None] * NP
        self.kt = [None] * NP
        self.et = [None] * NP
        self.rbt = [None] * NP
        self.psz = [None] * NP
        self.osb = [None] * NP

    def build(self):
        nc = self.nc
        with (
            self.tc.tile_pool(name="const", bufs=1) as constp,
            self.tc.tile_pool(name="vt", bufs=1) as vtp,
            self.tc.tile_pool(name="qk", bufs=2) as qkp,
            self.tc.tile_pool(name="et", bufs=2) as etp,
            self.tc.tile_pool(name="rz", bufs=2) as rzp,
            self.tc.tile_pool(name="rb", bufs=2) as rbp,
            self.tc.tile_pool(name="ob", bufs=2) as obp,
            self.tc.tile_pool(name="psA", bufs=1, space="PSUM") as psA,
            self.tc.tile_pool(name="psB", bufs=1, space="PSUM") as psB,
            self.tc.tile_pool(name="psPV", bufs=3, space="PSUM") as psPV,
            self.tc.tile_pool(name="psZ", bufs=1, space="PSUM") as psZ,
        ):
            self.qkp, self.etp, self.rzp, self.rbp, self.obp = (
                qkp, etp, rzp, rbp, obp,
            )
            self.psA, self.psB, self.psPV, self.psZ = psA, psB, psPV, psZ
            self.vtp = vtp

            self.ones_blk = constp.tile([128, 512], BF16)
            nc.vector.memset(self.ones_blk[:], 1.0)
            self.bias_ln = constp.tile([128, 1], F32)
            nc.vector.memset(self.bias_ln[:], float(np.log(ALPHA)))

            # PE p-state warmup: keep the PE busy from t=0 so the clock is
            # fully ramped (>3us continuous) when real matmuls arrive.
            wps = self.psA.tile([128, 1024], F32, tag="schunk")
            for i in range(6):
                nc.tensor.matmul(
                    wps[:, (i % 2) * 512:(i % 2) * 512 + 512],
                    lhsT=self.ones_blk[:, 0:128],
                    rhs=self.ones_blk[:],
                    start=True, stop=True,
                )

            self.stage_in(0)
            self.stage_in(1)
            self.stage_in_v()
            self.stage_qk(0)
            self.stage_zfin(0)
            self.stage_clip(0)
            self.stage_in(2)
            self.stage_qk(1, pv_pair=0)
            self.stage_zfin(1)
            self.stage_clip(1, drain_pair=0)
            self.stage_in(3)
            self.stage_qk(2, pv_pair=1)
            self.stage_zfin(2)
            self.stage_clip(2, drain_pair=1)
            self.stage_qk(3, pv_pair=2)
            self.stage_zfin(3)
            self.stage_clip(3, drain_pair=2, interleave_pv=True)
            self.emit_drains(3)
            self.finish_pv(3)

    def stage_in_v(self):
        # one wide DMA for all pairs' V: [S, NP*D] -> [128, (T, NP*D)]
        self.vt = self.vtp.tile([128, NT * NP * D], BF16)
        self.nc.sync.dma_start(
            out=self.vt[:].rearrange("p (t x) -> p t x", x=NP * D),
            in_=self.v_d.rearrange("(t p) x -> p t x", p=128),
        )

    def stage_in(self, j):
        nc = self.nc
        qt = self.qkp.tile([128, S], BF16, tag="qt")
        kt = self.qkp.tile([128, S], BF16, tag="kt")
        nc.sync.dma_start(out=kt[:], in_=self.kt_d[j])
        nc.sync.dma_start(out=qt[:], in_=self.qt_d[j])
        self.qt[j], self.kt[j] = qt, kt

    def _vblk(self, j, kk):
        off = (kk * NP + j) * D
        return self.vt[:, off:off + D]

    def stage_qk(self, j, pv_pair=None):
        """QK^T strips packed into alternating PSUM chunks + exp + inline
        Pool diag masks; the mini-Z matmuls go AFTER all chunks so they
        never block the PE queue mid-stream. Optionally interleaves the
        previous pair's clip/PV steps between chunks as PE filler."""
        nc = self.nc
        qt, kt = self.qt[j], self.kt[j]
        et = self.etp.tile([128, STREAM], BF16, tag="et")
        psz = self.psZ.tile([128, NT], F32, tag="zt")
        self.et[j] = et
        self.psz[j] = psz

        # fillers: previous pair's PV groups at LATE chunk slots, by which
        # time its clip prefixes are done on DVE so the PE queue never
        # blocks on the clip (drains are emitted later, after zfin(j))
        fillers = {}
        if pv_pair is not None:
            p = pv_pair
            fillers = {
                11: [lambda: self.emit_pv_part(p, 0, 0, 3, True, True)],
                12: [lambda: self.emit_pv_part(p, 1, 0, 3, True, False)],
                13: [lambda: self.emit_pv_part(p, 1, 4, 7, False, True),
                     lambda: self.emit_drain(p, 0, "act"),
                     lambda: self.emit_pv_part(p, 2, 0, 7, True, False)],
                15: [lambda: self.emit_pv_part(p, 2, 8, 11, False, True),
                     lambda: self.emit_drain(p, 1, "act"),
                     lambda: self.emit_pv_part(p, 3, 0, 11, True, False)],
            }

        for ci, (lo, hi, tog) in enumerate(CHUNKS):
            pool = self.psA if tog == 0 else self.psB
            ps = pool.tile([128, 1024], F32, tag="schunk")
            # matmul pieces: split at strip boundaries and 512-grid of tile
            pos = lo
            while pos < hi:
                kk = _strip_of(pos)
                strip_end = OFF_STRIP[kk] + W_STRIP[kk]
                seg_end = min(hi, strip_end, lo + ((pos - lo) // 512 + 1) * 512)
                qa = 128 * kk + (pos - OFF_STRIP[kk])
                qb = qa + (seg_end - pos)
                nc.tensor.matmul(
                    ps[:, pos - lo:seg_end - lo],
                    lhsT=kt[:, 128 * kk:128 * kk + 128],
                    rhs=qt[:, qa:qb],
                    start=True, stop=True,
                )
                pos = seg_end
            nc.scalar.activation(
                et[:, lo:hi],
                ps[:, 0:hi - lo],
                mybir.ActivationFunctionType.Exp,
                scale=float(INV_SQRT_D),
                bias=self.bias_ln[:],
            )
            # Pool-only diagonal masks as their chunk lands
            for kk in range(NT):
                dlo = OFF_STRIP[kk]
                if lo <= dlo and dlo + 128 <= hi:
                    nc.gpsimd.affine_select(
                        out=et[:, dlo:dlo + 128],
                        in_=et[:, dlo:dlo + 128],
                        compare_op=mybir.AluOpType.is_ge,
                        fill=0.0,
                        base=0,
                        pattern=[[1, 128]],
                        channel_multiplier=-1,
                    )
            for fn in fillers.pop(ci, ()):
                fn()
        # mini-Z: Z^T column T accumulates E over strips 0..T at q-tile T
        for T in range(NT):
            for k2 in range(T + 1):
                blk = OFF_STRIP[k2] + 128 * (T - k2)
                nc.tensor.matmul(
                    psz[:, T:T + 1],
                    lhsT=et[:, blk:blk + 128],
                    rhs=self.ones_blk[:, 0:1],
                    start=(k2 == 0), stop=(k2 == T),
                )
        for ci in sorted(fillers):
            for fn in fillers.pop(ci):
                fn()
        if pv_pair is not None:
            self.emit_pv_part(pv_pair, 3, 12, 15, False, True)

    def stage_zfin(self, j):
        """Reciprocal + transpose to row + partition broadcast -> rbt."""
        nc = self.nc
        rt32 = self.rzp.tile([128, 16], F32, tag="rt32")
        rt = self.rzp.tile([128, 32], BF16, tag="rt")
        rrow = self.rzp.tile([32, 128], BF16, tag="rrow")
        rbt = self.rbp.tile([128, S], BF16, tag="rbt")
        self.rbt[j] = rbt
        # r = (ALPHA/0.03)/Z'  (constant folded into the bf16 cast below)
        nc.vector.reciprocal(rt32[:, 0:NT], self.psz[j][:, 0:NT])
        nc.vector.tensor_scalar_mul(rt[:, 0:NT], rt32[:, 0:NT], float(ALPHA / -GAMMA))
        nc.vector.memset(rt[:, NT:32], 1.0)
        for i in range(4):
            nc.vector.transpose(
                out=rrow[0:32, 32 * i:32 * i + 32],
                in_=rt[32 * i:32 * i + 32, 0:32],
            )
        # gather the 16 transposed rows into one q-major row on partition 0,
        # then broadcast it across all partitions in a single Pool op
        row0 = self.rzp.tile([1, S], BF16, tag="row0")
        nc.sync.dma_start(out=row0[0:1, 0:S], in_=rrow[0:NT, 0:128])
        nc.gpsimd.partition_broadcast(rbt[:], row0[0:1, 0:S], channels=128)

    def stage_clip(self, j, drain_pair=None, interleave_pv=False):
        """Ehat = E * rbt per strip and the dual-op clamp, in 4 prefix
        pieces, on DVE concurrently with qk(j+1). The previous pair's
        remaining PSUM drains (groups 2,3) execute up front."""
        nc = self.nc
        et, rbt = self.et[j], self.rbt[j]
        if drain_pair is not None:
            self.emit_drain(drain_pair, 2)
            self.emit_drain(drain_pair, 3)
            self.finish_pv(drain_pair)
        for g in range(4):
            kmax = 4 * g + 3
            for kk in range(4 * g, kmax + 1):
                lo, w = OFF_STRIP[kk], W_STRIP[kk]
                nc.vector.tensor_tensor(
                    et[:, lo:lo + w],
                    et[:, lo:lo + w],
                    rbt[:, 128 * kk:S],
                    mybir.AluOpType.mult,
                )
            clo = OFF_STRIP[4 * g]
            chi_ = OFF_STRIP[kmax] + W_STRIP[kmax]
            nc.vector.tensor_scalar(
                et[:, clo:chi_], et[:, clo:chi_], float(CHI), 1.0,
                mybir.AluOpType.min, mybir.AluOpType.max,
            )
            if interleave_pv:
                if g == 0:
                    self.emit_pv_part(j, 0, 0, 3, True, True)
                    self.emit_pv_part(j, 1, 0, 3, True, False)
                elif g == 1:
                    self.emit_pv_part(j, 1, 4, 7, False, True)
                    self.emit_drain(j, 0, "act")
                    self.emit_pv_part(j, 2, 0, 7, True, False)
                elif g == 2:
                    self.emit_pv_part(j, 2, 8, 11, False, True)
                    self.emit_drain(j, 1, "act")
                    self.emit_pv_part(j, 3, 0, 11, True, False)
                else:
                    self.emit_pv_part(j, 3, 12, 15, False, True)

    def emit_pv_part(self, j, g, kk_lo, kk_hi, start, stop):
        """PV matmuls for group g (cols [512g,512g+512)) over strips
        kk_lo..kk_hi, accumulating into the group's psum tile."""
        nc = self.nc
        et = self.et[j]
        if self.osb[j] is None:
            self.osb[j] = self.obp.tile([128, S], F32, tag="osb", name="osb")
        glo, ghi = 512 * g, 512 * (g + 1)
        self._pvps = getattr(self, "_pvps", {})
        if (j, g) not in self._pvps:
            self._pvps[(j, g)] = self.psPV.tile([128, 512], F32, tag="pv", name="pv")
        ps = self._pvps[(j, g)]
        for kk in range(kk_lo, kk_hi + 1):
            qlo = max(glo, 128 * kk)
            src = OFF_STRIP[kk] + (qlo - 128 * kk)
            nc.tensor.matmul(
                ps[:, qlo - glo:512],
                lhsT=self._vblk(j, kk),
                rhs=et[:, src:src + (ghi - qlo)],
                start=(kk == kk_lo and start), stop=(kk == kk_hi and stop),
            )

    def emit_pv_group(self, j, g):
        self.emit_pv_part(j, g, 0, 4 * g + 3, True, True)

    def emit_drain(self, j, g, eng="dve"):
        done = getattr(self, "_drained", set())
        self._drained = done
        if (j, g) in done or (j, g) not in getattr(self, "_pvps", {}):
            return
        done.add((j, g))
        ps = self._pvps.pop((j, g))
        dst = self.osb[j][:, 512 * g:512 * (g + 1)]
        if eng == "act":
            self.nc.scalar.copy(dst, ps[:, 0:512])
        else:
            self.nc.vector.tensor_scalar_add(dst, ps[:, 0:512], 0.0)

    def emit_drains(self, j):
        for g in range(4):
            self.emit_drain(j, g)

    def finish_pv(self, j):
        self.nc.sync.dma_start(out=self.ot_d[j], in_=self.osb[j][:])


_NC_CACHE = None


def _get_program():
    global _NC_CACHE
    if _NC_CACHE is None:
        _NC_CACHE = build_core_program()
    return _NC_CACHE


def kernel(query_states, key_states, value_states, batch_size, q_length, kv_length):
    assert int(batch_size) == B and int(q_length) == S and int(kv_length) == S
    qf = np.asarray(query_states, dtype=np.float32).reshape(B, S, H, D)
    kf = np.asarray(key_states, dtype=np.float32).reshape(B, S, H, D)
    vf = np.asarray(value_states, dtype=np.float32).reshape(B, S, H, D)

    nc = _get_program()

    in_maps = []
    s_host = []  # per core: [NP, NT, D] causal-prefix sums of bf16 V
    for c in range(N_CORES):
        b = c // (N_CORES // B)
        h0 = NP * (c % (N_CORES // B))
        qb = qf[b, :, h0:h0 + NP, :].astype(ml_dtypes.bfloat16)  # [S, NP, D]
        kb = kf[b, :, h0:h0 + NP, :].astype(ml_dtypes.bfloat16)
        vb = vf[b, :, h0:h0 + NP, :].astype(ml_dtypes.bfloat16)
        in_maps.append(
            {
                "qt": np.ascontiguousarray(qb.transpose(1, 2, 0)),  # [NP,D,S]
                "kt": np.ascontiguousarray(kb.transpose(1, 2, 0)),
                "v": np.ascontiguousarray(vb.reshape(S, NP * D)),
            }
        )
        # S_T[d] = sum of V over k-tiles 0..T (f32 accumulation of bf16 V)
        vt32 = vb.astype(np.float32).reshape(NT, 128, NP, D)
        s_host.append(np.cumsum(vt32.sum(axis=1), axis=0).transpose(1, 0, 2))

    res = run_bass_kernel_spmd(nc, in_maps, list(range(N_CORES)))

    out = np.empty((B, S, H, D), dtype=np.float32)
    for c in range(N_CORES):
        b = c // (N_CORES // B)
        h0 = NP * (c % (N_CORES // B))
        ot = np.asarray(res.results[c]["ot"])  # [NP, D, S] = PV(t2)
        for jj in range(NP):
            pv = ot[jj].T.reshape(NT, 128, D)  # [T, q, D]
            pv = pv - s_host[c][jj][:, None, :]
            out[b, :, h0 + jj, :] = (-GAMMA) * pv.reshape(S, D)
    return out.reshape(B * S, H, D)


# revision 21
# speedup vs baseline: 1.2309x; 1.0169x over previous
"""Trainium2 Bass kernel for causal self-attention with clipped softmax.

Problem (hardcoded): B=2, S=2048, H=16, D=128, fp32 inputs.
    scores = (Q @ K^T) / sqrt(D), causal mask, p = softmax(scores)
    p = clip(1.06*p - 0.03, 0, 1)            # ZETA=1.03, GAMMA=-0.03
    out = p @ V

Sharding: 32 (batch, head) pairs -> 4 per core across 8 cores (tensor
parallel over heads + data parallel over batch). No cross-core comms.

Per-core device algorithm (transposed-scores layout, bf16 matmuls):
  - Q^T, K^T are staged host-side as [NP, D, S] so no on-device transposes;
    V is staged pair-interleaved [S, NP*D] for wide DMA descriptors.
  - scoresT[k, q] strips (causal only) are packed into one contiguous
    "stream" per pair so each exp activation covers a full PSUM tile.
  - E = exp(s/sqrt(D) + ln 1.06), diagonal masked by affine_select.
  - Z^T[q, tile] via tiny-output matmuls: E-block stationary, ones column
    moving -> PSUM accumulates the per-tile softmax denominators.
  - r[q] = (1.06/0.03)/Z' via one Reciprocal activation; transposed to a
    row with DVE 32x32 stream transposes; broadcast across partitions by
    GPSIMD partition_broadcast -> rbt[k, q] tile.
  - Ehat = E * rbt (one DVE tensor_tensor pass), then ONE dual-op
    tensor_scalar clamp: t2 = max(min(Ehat, 34.333), 1.0).
    Identity: clip(1.06p - 0.03, 0, 1) = 0.03*(t2 - 1), and masked
    positions (Ehat=0 -> t2=1) plus the -1 offset are together equal to
    the causal-prefix V sums, which the HOST subtracts after the fact:
      out[d, q in tile T] = 0.03*(PV(t2)[d, q] - S_T[d]),
      S_T[d] = sum of V[k, d] over all k-tiles <= T.
  - PV(t2) accumulated in PSUM over k-tiles, drained f32 to SBUF, DMA'd
    out as [NP, D, S] f32; host applies the S_T correction + 0.03 scale.
"""

import ml_dtypes
import numpy as np

import concourse.bass as bass
import concourse.mybir as mybir
import concourse.tile as tile
from concourse import bacc
from concourse.bass_utils import run_bass_kernel_spmd

B = 2
S = 2048
H = 16
D = 128
N_CORES = 8
NP = H * B // N_CORES  # (b,h) pairs per core = 4
NT = S // 128  # 128-col tiles along sequence = 16
INV_SQRT_D = 1.0 / np.sqrt(np.float64(D))
ZETA = 1.03
GAMMA = -0.03
ALPHA = ZETA - GAMMA  # 1.06
CHI = (1.0 - GAMMA) / (-GAMMA)  # 34.3333: upper clamp for Ehat
STREAM = S * NT - 64 * NT * (NT - 1)  # 17408 packed causal columns

F32 = mybir.dt.float32
BF16 = mybir.dt.bfloat16

# strip kk (k-tile) covers q in [128*kk, S); stream offset of each strip
W_STRIP = [S - 128 * kk for kk in range(NT)]
OFF_STRIP = [0] * NT
for _kk in range(1, NT):
    OFF_STRIP[_kk] = OFF_STRIP[_kk - 1] + W_STRIP[_kk - 1]

# exp/psum chunking of the packed stream: alternate 1536/1024 tiles
CHUNKS = []  # (stream_lo, stream_hi, which_pool)
_pos = 0
_tog = 0
while _pos < STREAM:
    hi = min(_pos + 1024, STREAM)
    CHUNKS.append((_pos, hi, _tog))
    _pos = hi
    _tog ^= 1


def _strip_of(pos):
    for kk in range(NT - 1, -1, -1):
        if pos >= OFF_STRIP[kk]:
            return kk
    raise AssertionError


def build_core_program():
    nc = bacc.Bacc(
        "TRN2", target_bir_lowering=False, debug=False, num_devices=N_CORES
    )

    qt_d = nc.dram_tensor("qt", [NP, D, S], BF16, kind="ExternalInput").ap()
    kt_d = nc.dram_tensor("kt", [NP, D, S], BF16, kind="ExternalInput").ap()
    v_d = nc.dram_tensor("v", [S, NP * D], BF16, kind="ExternalInput").ap()
    ot_d = nc.dram_tensor("ot", [NP, D, S], F32, kind="ExternalOutput").ap()

    with tile.TileContext(nc) as tc:
        Builder(tc, qt_d, kt_d, v_d, ot_d).build()

    nc.compile()
    return nc


class Builder:
    def __init__(self, tc, qt_d, kt_d, v_d, ot_d):
        self.tc = tc
        self.nc = tc.nc
        self.qt_d, self.kt_d, self.v_d, self.ot_d = qt_d, kt_d, v_d, ot_d
        self.qt = [None] * NP
        self.kt = [None] * NP
        self.et = [None] * NP
        self.rbt = [None] * NP
        self.psz = [None] * NP
        self.osb = [None] * NP

    def build(self):
        nc = self.nc
        with (
            self.tc.tile_pool(name="const", bufs=1) as constp,
            self.tc.tile_pool(name="vt", bufs=1) as vtp,
            self.tc.tile_pool(name="qk", bufs=2) as qkp,
            self.tc.tile_pool(name="et", bufs=2) as etp,
            self.tc.tile_pool(name="rz", bufs=2) as rzp,
            self.tc.tile_pool(name="rb", bufs=2) as rbp,
            self.tc.tile_pool(name="ob", bufs=2) as obp,
            self.tc.tile_pool(name="psA", bufs=1, space="PSUM") as psA,
            self.tc.tile_pool(name="psB", bufs=1, space="PSUM") as psB,
            self.tc.tile_pool(name="psPV", bufs=3, space="PSUM") as psPV,
            self.tc.tile_pool(name="psZ", bufs=1, space="PSUM") as psZ,
        ):
            self.qkp, self.etp, self.rzp, self.rbp, self.obp = (
                qkp, etp, rzp, rbp, obp,
            )
            self.psA, self.psB, self.psPV, self.psZ = psA, psB, psPV, psZ
            self.vtp = vtp

            self.ones_blk = constp.tile([128, 512], BF16)
            nc.vector.memset(self.ones_blk[:], 1.0)
            self.bias_ln = constp.tile([128, 1], F32)
            nc.vector.memset(self.bias_ln[:], float(np.log(ALPHA)))

            # PE p-state warmup: keep the PE busy from t=0 so the clock is
            # fully ramped (>3us continuous) when real matmuls arrive.
            wps = self.psA.tile([128, 1024], F32, tag="schunk")
            for i in range(6):
                nc.tensor.matmul(
                    wps[:, (i % 2) * 512:(i % 2) * 512 + 512],
                    lhsT=self.ones_blk[:, 0:128],
                    rhs=self.ones_blk[:],
                    start=True, stop=True,
                )

            self.stage_in(0)
            self.stage_in(1)
            self.stage_in_v()

            # v6 software pipeline. Window w runs pair w's QK/exp stream on
            # PE+Act while DVE runs pair (w-1)'s clip; PV groups 0-2 of the
            # clipped pair land at late chunk slots, group 3 early in the
            # NEXT window (after the clip tail), drains at window starts.
            for w in range(NP):
                p = w - 1
                if p >= 1:
                    for g in range(3):
                        self.emit_drain(p - 1, g)
                if p >= 0:
                    self.stage_clip_head(p)
                self.stage_qk(w, pv_pair=p if p >= 0 else None)
                self.stage_zfin_dve(w)
                if p >= 0:
                    self.stage_clip_tail(p)
                self.stage_zfin_pool(w)
                if w + 2 < NP:
                    self.stage_in(w + 2)

            # tail: pair 2 final group + all of pair 3's clip/PV
            for g in range(3):
                self.emit_drain(2, g)
            self.emit_pv_group(2, 3)
            self.emit_drain(2, 3)
            self.finish_pv(2)
            self.stage_clip_head(3, interleave_pv=True)
            self.stage_clip_tail(3)
            self.emit_drain(3, 0)
            self.emit_pv_group(3, 3)
            for g in range(1, 4):
                self.emit_drain(3, g)
            self.finish_pv(3)

    def stage_in_v(self):
        # one wide DMA for all pairs' V: [S, NP*D] -> [128, (T, NP*D)]
        self.vt = self.vtp.tile([128, NT * NP * D], BF16)
        self.nc.sync.dma_start(
            out=self.vt[:].rearrange("p (t x) -> p t x", x=NP * D),
            in_=self.v_d.rearrange("(t p) x -> p t x", p=128),
        )

    def stage_in(self, j):
        nc = self.nc
        qt = self.qkp.tile([128, S], BF16, tag="qt")
        kt = self.qkp.tile([128, S], BF16, tag="kt")
        nc.sync.dma_start(out=kt[:], in_=self.kt_d[j])
        nc.sync.dma_start(out=qt[:], in_=self.qt_d[j])
        self.qt[j], self.kt[j] = qt, kt

    def _vblk(self, j, kk):
        off = (kk * NP + j) * D
        return self.vt[:, off:off + D]

    def stage_qk(self, j, pv_pair=None):
        """QK^T strips packed into alternating PSUM chunks + exp + inline
        Pool diag masks; the mini-Z matmuls go AFTER all chunks so they
        never block the PE queue mid-stream. Optionally interleaves the
        previous pair's clip/PV steps between chunks as PE filler."""
        nc = self.nc
        qt, kt = self.qt[j], self.kt[j]
        et = self.etp.tile([128, STREAM], BF16, tag="et")
        psz = self.psZ.tile([128, NT], F32, tag="zt")
        self.et[j] = et
        self.psz[j] = psz

        # fillers: previous pair's PV groups at LATE chunk slots, by which
        # time its clip prefixes are done on DVE so the PE queue never
        # blocks on the clip (drains are emitted later, after zfin(j))
        fillers = {}
        if pv_pair is not None:
            p = pv_pair
            fillers = {
                11: [lambda: self.emit_pv_group(p, 0)],
                13: [lambda: self.emit_pv_group(p, 1)],
                15: [lambda: self.emit_pv_group(p, 2)],
            }
            if p >= 1:
                fillers[2] = [lambda: self.emit_pv_group(p - 1, 3)]
                fillers[4] = [lambda: self.emit_drain(p - 1, 3, "act"),
                              lambda: self.finish_pv(p - 1)]

        for ci, (lo, hi, tog) in enumerate(CHUNKS):
            pool = self.psA if tog == 0 else self.psB
            ps = pool.tile([128, 1024], F32, tag="schunk")
            # matmul pieces: split at strip boundaries and 512-grid of tile
            pos = lo
            while pos < hi:
                kk = _strip_of(pos)
                strip_end = OFF_STRIP[kk] + W_STRIP[kk]
                seg_end = min(hi, strip_end, lo + ((pos - lo) // 512 + 1) * 512)
                qa = 128 * kk + (pos - OFF_STRIP[kk])
                qb = qa + (seg_end - pos)
                nc.tensor.matmul(
                    ps[:, pos - lo:seg_end - lo],
                    lhsT=kt[:, 128 * kk:128 * kk + 128],
                    rhs=qt[:, qa:qb],
                    start=True, stop=True,
                )
                pos = seg_end
            nc.scalar.activation(
                et[:, lo:hi],
                ps[:, 0:hi - lo],
                mybir.ActivationFunctionType.Exp,
                scale=float(INV_SQRT_D),
                bias=self.bias_ln[:],
            )
            # Pool-only diagonal masks as their chunk lands
            for kk in range(NT):
                dlo = OFF_STRIP[kk]
                if lo <= dlo and dlo + 128 <= hi:
                    nc.gpsimd.affine_select(
                        out=et[:, dlo:dlo + 128],
                        in_=et[:, dlo:dlo + 128],
                        compare_op=mybir.AluOpType.is_ge,
                        fill=0.0,
                        base=0,
                        pattern=[[1, 128]],
                        channel_multiplier=-1,
                    )
            for fn in fillers.pop(ci, ()):
                fn()
        # mini-Z: Z^T column T accumulates E over strips 0..T at q-tile T
        for T in range(NT):
            for k2 in range(T + 1):
                blk = OFF_STRIP[k2] + 128 * (T - k2)
                nc.tensor.matmul(
                    psz[:, T:T + 1],
                    lhsT=et[:, blk:blk + 128],
                    rhs=self.ones_blk[:, 0:1],
                    start=(k2 == 0), stop=(k2 == T),
                )
        for ci in sorted(fillers):
            for fn in fillers.pop(ci):
                fn()
        if pv_pair is not None:
            self.emit_pv_part(pv_pair, 3, 12, 15, False, True)

    def stage_zfin_dve(self, j):
        """Reciprocal + scale-cast + 32x32 stream transposes (DVE)."""
        nc = self.nc
        rt32 = self.rzp.tile([128, 16], F32, tag="rt32")
        rt = self.rzp.tile([128, 32], BF16, tag="rt")
        rrow = self.rzp.tile([32, 128], BF16, tag="rrow")
        self._rrow = getattr(self, "_rrow", {})
        self._rrow[j] = rrow
        nc.vector.reciprocal(rt32[:, 0:NT], self.psz[j][:, 0:NT])
        nc.vector.tensor_scalar_mul(rt[:, 0:NT], rt32[:, 0:NT], float(ALPHA / -GAMMA))
        nc.vector.memset(rt[:, NT:32], 1.0)
        for i in range(4):
            nc.vector.transpose(
                out=rrow[0:32, 32 * i:32 * i + 32],
                in_=rt[32 * i:32 * i + 32, 0:32],
            )

    def stage_zfin_pool(self, j):
        """Row gather DMA + partition broadcast -> rbt (SP/Pool)."""
        nc = self.nc
        rrow = self._rrow[j]
        rbt = self.rbp.tile([128, S], BF16, tag="rbt")
        self.rbt[j] = rbt
        row0 = self.rzp.tile([1, S], BF16, tag="row0")
        nc.sync.dma_start(out=row0[0:1, 0:S], in_=rrow[0:NT, 0:128])
        nc.gpsimd.partition_broadcast(rbt[:], row0[0:1, 0:S], channels=128)

    def stage_clip_head(self, j, interleave_pv=False):
        """Ehat = E * rbt for strips 0-11 + clamp pieces B0..B2 (DVE)."""
        nc = self.nc
        et, rbt = self.et[j], self.rbt[j]
        for g in range(3):
            kmax = 4 * g + 3
            for kk in range(4 * g, kmax + 1):
                lo, w = OFF_STRIP[kk], W_STRIP[kk]
                nc.vector.tensor_tensor(
                    et[:, lo:lo + w],
                    et[:, lo:lo + w],
                    rbt[:, 128 * kk:S],
                    mybir.AluOpType.mult,
                )
            clo = OFF_STRIP[4 * g]
            chi_ = OFF_STRIP[kmax] + W_STRIP[kmax]
            nc.vector.tensor_scalar(
                et[:, clo:chi_], et[:, clo:chi_], float(CHI), 1.0,
                mybir.AluOpType.min, mybir.AluOpType.max,
            )
            if interleave_pv:
                self.emit_pv_group(j, g)

    def stage_clip_tail(self, j):
        """Strips 12-15 + clamp piece B3; emitted after the NEXT pair's
        zfin DVE ops so the reciprocal chain is not queued behind it."""
        nc = self.nc
        et, rbt = self.et[j], self.rbt[j]
        for kk in range(12, NT):
            lo, w = OFF_STRIP[kk], W_STRIP[kk]
            nc.vector.tensor_tensor(
                et[:, lo:lo + w],
                et[:, lo:lo + w],
                rbt[:, 128 * kk:S],
                mybir.AluOpType.mult,
            )
        clo = OFF_STRIP[12]
        nc.vector.tensor_scalar(
            et[:, clo:STREAM], et[:, clo:STREAM], float(CHI), 1.0,
            mybir.AluOpType.min, mybir.AluOpType.max,
        )

    def emit_pv_part(self, j, g, kk_lo, kk_hi, start, stop):
        """PV matmuls for group g (cols [512g,512g+512)) over strips
        kk_lo..kk_hi, accumulating into the group's psum tile."""
        nc = self.nc
        et = self.et[j]
        if self.osb[j] is None:
            self.osb[j] = self.obp.tile([128, S], F32, tag="osb", name="osb")
        glo, ghi = 512 * g, 512 * (g + 1)
        self._pvps = getattr(self, "_pvps", {})
        if (j, g) not in self._pvps:
            self._pvps[(j, g)] = self.psPV.tile([128, 512], F32, tag="pv", name="pv")
        ps = self._pvps[(j, g)]
        for kk in range(kk_lo, kk_hi + 1):
            qlo = max(glo, 128 * kk)
            src = OFF_STRIP[kk] + (qlo - 128 * kk)
            nc.tensor.matmul(
                ps[:, qlo - glo:512],
                lhsT=self._vblk(j, kk),
                rhs=et[:, src:src + (ghi - qlo)],
                start=(kk == kk_lo and start), stop=(kk == kk_hi and stop),
            )

    def emit_pv_group(self, j, g):
        self.emit_pv_part(j, g, 0, 4 * g + 3, True, True)

    def emit_drain(self, j, g, eng="dve"):
        done = getattr(self, "_drained", set())
        self._drained = done
        if (j, g) in done or (j, g) not in getattr(self, "_pvps", {}):
            return
        done.add((j, g))
        ps = self._pvps.pop((j, g))
        dst = self.osb[j][:, 512 * g:512 * (g + 1)]
        if eng == "act":
            self.nc.scalar.copy(dst, ps[:, 0:512])
        else:
            self.nc.vector.tensor_scalar_add(dst, ps[:, 0:512], 0.0)

    def emit_drains(self, j):
        for g in range(4):
            self.emit_drain(j, g)

    def finish_pv(self, j):
        self.nc.sync.dma_start(out=self.ot_d[j], in_=self.osb[j][:])


_NC_CACHE = None


def _get_program():
    global _NC_CACHE
    if _NC_CACHE is None:
        _NC_CACHE = build_core_program()
    return _NC_CACHE


def kernel(query_states, key_states, value_states, batch_size, q_length, kv_length):
    assert int(batch_size) == B and int(q_length) == S and int(kv_length) == S
    qf = np.asarray(query_states, dtype=np.float32).reshape(B, S, H, D)
    kf = np.asarray(key_states, dtype=np.float32).reshape(B, S, H, D)
    vf = np.asarray(value_states, dtype=np.float32).reshape(B, S, H, D)

    nc = _get_program()

    in_maps = []
    s_host = []  # per core: [NP, NT, D] causal-prefix sums of bf16 V
    for c in range(N_CORES):
        b = c // (N_CORES // B)
        h0 = NP * (c % (N_CORES // B))
        qb = qf[b, :, h0:h0 + NP, :].astype(ml_dtypes.bfloat16)  # [S, NP, D]
        kb = kf[b, :, h0:h0 + NP, :].astype(ml_dtypes.bfloat16)
        vb = vf[b, :, h0:h0 + NP, :].astype(ml_dtypes.bfloat16)
        in_maps.append(
            {
                "qt": np.ascontiguousarray(qb.transpose(1, 2, 0)),  # [NP,D,S]
                "kt": np.ascontiguousarray(kb.transpose(1, 2, 0)),
                "v": np.ascontiguousarray(vb.reshape(S, NP * D)),
            }
        )
        # S_T[d] = sum of V over k-tiles 0..T (f32 accumulation of bf16 V)
        vt32 = vb.astype(np.float32).reshape(NT, 128, NP, D)
        s_host.append(np.cumsum(vt32.sum(axis=1), axis=0).transpose(1, 0, 2))

    res = run_bass_kernel_spmd(nc, in_maps, list(range(N_CORES)))

    out = np.empty((B, S, H, D), dtype=np.float32)
    for c in range(N_CORES):
        b = c // (N_CORES // B)
        h0 = NP * (c % (N_CORES // B))
        ot = np.asarray(res.results[c]["ot"])  # [NP, D, S] = PV(t2)
        for jj in range(NP):
            pv = ot[jj].T.reshape(NT, 128, D)  # [T, q, D]
            pv = pv - s_host[c][jj][:, None, :]
            out[b, :, h0 + jj, :] = (-GAMMA) * pv.reshape(S, D)
    return out.reshape(B * S, H, D)
